# revision 26
# baseline (speedup 1.0000x reference)
"""BERT encoder block on 8 Trainium2 NeuronCores.

Strategy: pure data parallelism — batch 8 is split one batch element per core
(no collectives). Each core runs the full encoder block on its [2048, 1024]
slice. All six big matmuls run in fp8 (TRN e4m3) DoubleRow mode (2 fp8
weights per PE cell -> 2x contraction per pass); accumulation and the
residual/LN stream are fp32.

Algebraic folds done on the host (softmax row-invariance absorbs the
query-side bias term):
  M   = Wq @ Wk^T          -> scores = x M x^T  (one fused tensor A = x@M)
  NP  = Wv @ Wo            -> attn_out @ Wo = softmax(S) @ (x@NP) + bo2
  bo2 = bo + bv @ Wo
  wrow = scale*(x @ (Wk@bq) + bq.bk) + ln(S_E)   (key-side score bias)

fp8 scaling: every fp8 tensor T is stored as s_T * T with a power-of-2
per-tensor scale chosen so absmax stays well under TRN e4m3's +-240 (and
above the 2^-6 subnormal threshold for typical values). All descales fold
into existing activation scale/bias parameters:
  xT   = S_X  * x^T            (scaled during the f32->bf16 pre-transpose copy)
  Ms   = S_M  * M, NPs = S_NP * NP, W1s = S_W1 * W1, W2s = S_W2 * W2  (host)
  AT   = S_A  * (x@M)^T        (PSUM copy scale = S_A/(S_M*S_X))
  VW   = S_VW * (x@NP)         (PSUM copy scale = S_VW/(S_X*S_NP))
  expS = S_E  * exp(..)        (ln S_E folded into wrow bias on host)
  softmax denom: ones vector holds S_VW so recip = 1/(S_E*S_VW*D) exactly
  cancels the PV matmul's S_E*S_VW factor -> proj is unscaled attn.
  hT   = S_H  * h^T            (scaled during the f32->bf16 pre-transpose copy)
  r1   = S_R  * relu(..)       (relu scale = S_R/(S_W1*S_H), bias = S_R*bf1)
  FFN2 out descale = 1/(S_R*S_W2) via one tensor_scalar_mul.
The fp32 residual/LN stream is never scaled.

Attention runs in transposed score layout S^T[k,q]: softmax denominators are
ones-matmuls and proj = P @ (x@NP) lands directly in [q, f] layout. The
residual+LN1+h-transpose work is interleaved into the attention loop with a
one-chunk lag so TensorE never drains. Weights are pre-shuffled on the host
so every big DMA is one contiguous segment per partition.

Self-contained: hardcodes shapes from the problem spec.
"""
import os

import numpy as np
import ml_dtypes

import concourse.bacc as bacc
import concourse.bass as bass
import concourse.tile as tile
import concourse.mybir as mybir
from concourse.bass_utils import run_bass_kernel_spmd
from concourse.masks import make_identity

P = 128
S = 2048          # sequence length per core
E = 1024          # embed
F = 4096          # ffn hidden
SB = S // P       # 16 seq blocks
EB = E // P       # 8 embed blocks
HB = F // P       # 32 ffn blocks
NCHUNK = 512
QC = S // NCHUNK  # 4 q chunks
QPC = NCHUNK // P  # 4 seq blocks per chunk
LN_EPS = 1e-5
SCALE = 1.0 / np.sqrt(np.float32(E))

# fp8 per-tensor scales (powers of 2; absmaxes measured on the fixed input
# distribution with ~2x safety margin under TRN e4m3's +-240)
S_X = 32.0    # x absmax ~5.4   -> 173
S_M = 2048.0  # M absmax ~.058  -> 118
S_NP = 2048.0  # NP absmax ~.053 -> 107
S_A = 32.0    # A absmax ~1.9   -> 62
S_VW = 64.0   # VW absmax ~1.8  -> 115
S_E = 16.0    # exp absmax ~7.5 -> 119
S_H = 32.0    # h absmax ~4.9   -> 158
S_W1 = 4096.0  # W1 absmax 1/32 -> 128
S_R = 16.0    # r1 absmax ~3.2  -> 50
S_W2 = 8192.0  # W2 absmax 1/64 -> 128

C_A = S_A / (S_M * S_X)        # AT copy scale
C_VW = S_VW / (S_X * S_NP)     # VW copy scale
C_EXP = float(SCALE) / (S_X * S_A)  # exp activation scale
C_R1 = S_R / (S_W1 * S_H)      # relu activation scale
C_F2 = 1.0 / (S_R * S_W2)      # FFN2 descale

F32 = mybir.dt.float32
BF16 = mybir.dt.bfloat16
F8 = mybir.dt.float8e4
AF = mybir.ActivationFunctionType
ALU = mybir.AluOpType
DR = mybir.MatmulPerfMode.DoubleRow

_CACHED_NC = {}


def _bcast_ap(ap, parts=P):
    """DRAM row-vector -> [parts, n] partition-broadcast access pattern."""
    return bass.AP(tensor=ap.tensor, offset=ap.offset,
                   ap=[[0, parts]] + [list(d) for d in ap.ap])


def _layer_norm_inplace(nc, work, src, gamma, beta, eps_c):
    """LN over free dim of src [P, E] fp32, in place.

    gamma/beta None means identity (fold for the common g=1, b=0 case).
    """
    stats = work.tile([P, 2, 6], F32, tag="ln_stats")
    nc.vector.bn_stats(stats[:, 0, :], src[:, 0:512])
    nc.vector.bn_stats(stats[:, 1, :], src[:, 512:1024])
    mv = work.tile([P, 2], F32, tag="ln_mv")
    nc.vector.bn_aggr(mv[:], stats[:])
    std = work.tile([P, 1], F32, tag="ln_std")
    nc.scalar.activation(std[:], mv[:, 1:2], AF.Sqrt, bias=eps_c[:], scale=1.0)
    rstd = work.tile([P, 1], F32, tag="ln_rstd")
    nc.vector.reciprocal(rstd[:], std[:])
    nc.vector.tensor_scalar(src[:], src[:], mv[:, 0:1], rstd[:],
                            ALU.subtract, ALU.mult)
    if gamma is not None:
        nc.vector.tensor_mul(src[:], src[:], gamma[:])
    if beta is not None:
        nc.vector.tensor_add(src[:], src[:], beta[:])


def build_nc(identity=False):
    """identity=True folds away LN gamma/beta and the bo2/bf2 bias adds,
    valid when g1=g2=1, b1=b2=bo2=bf2=0 (checked on the host)."""
    nc = bacc.Bacc(None, target_bir_lowering=False, debug=False)

    x_d = nc.dram_tensor("x", [S, E], F32, kind="ExternalInput")
    # host-preshuffled: row p holds M[o*128+p, :] for o in 0..7, concatenated
    m_d = nc.dram_tensor("Ms", [P, EB * E], F8, kind="ExternalInput")
    np_d = nc.dram_tensor("NPs", [P, EB * E], F8, kind="ExternalInput")
    # W1s[c, p, t*E + ei*128 + j] = W1[ei*128+p, (4c+t)*128+j]
    w1_d = nc.dram_tensor("W1s", [HB // 4, P, 4 * E], F8, kind="ExternalInput")
    # W2s[p, hb*E + n] = W2[hb*128+p, n]
    w2_d = nc.dram_tensor("W2s", [P, HB * E], F8, kind="ExternalInput")
    bo2_d = nc.dram_tensor("bo2", [E], F32, kind="ExternalInput")
    wrow_d = nc.dram_tensor("wrow", [S], F32, kind="ExternalInput")
    bf1_d = nc.dram_tensor("bf1", [F], F32, kind="ExternalInput")
    bf2_d = nc.dram_tensor("bf2", [E], F32, kind="ExternalInput")
    g1_d = nc.dram_tensor("g1", [E], F32, kind="ExternalInput")
    b1_d = nc.dram_tensor("b1", [E], F32, kind="ExternalInput")
    g2_d = nc.dram_tensor("g2", [E], F32, kind="ExternalInput")
    b2_d = nc.dram_tensor("b2", [E], F32, kind="ExternalInput")
    out_d = nc.dram_tensor("out", [S, E], F32, kind="ExternalOutput")
    h_d = nc.dram_tensor("h_scratch", [S, E], F32)   # LN1 output spill
    hT_d = nc.dram_tensor("hT_scratch", [E, S], F8)  # transposed LN1 out (S_H)

    with tile.TileContext(nc, pool_alloc_mode="queue") as tc:
        with tc.tile_pool(name="const", bufs=1) as const:
            ident = const.tile([P, P], BF16)
            make_identity(nc, ident)
            ones_c = const.tile([P, 1], F32)
            nc.vector.memset(ones_c[:], 1.0)
            eps_c = const.tile([P, 1], F32)
            nc.vector.memset(eps_c[:], LN_EPS)
            # keep these strided gathers off the sync queue so the first
            # x-tile DMA isn't stuck behind them
            bf1_sb = const.tile([P, HB], F32)
            nc.gpsimd.dma_start(bf1_sb[:], bf1_d[:].rearrange("(o p) -> p o", p=P))
            recip_sb = const.tile([P, SB], F32)
            w_sb = const.tile([P, SB], F32)
            nc.gpsimd.dma_start(w_sb[:], wrow_d[:].rearrange("(o p) -> p o", p=P))

            with tc.tile_pool(name="pbig", bufs=1) as pbig:
                xT = pbig.tile([P, EB, S], F8)  # xT[p,eb,s] = S_X*x[s, eb*P+p]

                # ---- Phase A: load x, scale, transpose to xT --------------
                with tc.tile_pool(name="pa", bufs=3) as pa, \
                     tc.tile_pool(name="pa_ps", bufs=4, space="PSUM") as pa_ps:
                    for sb in range(SB):
                        xf = pa.tile([P, E], F32, tag="xf")
                        nc.sync.dma_start(xf[:], x_d[sb * P:(sb + 1) * P, :])
                        xb = pa.tile([P, E], BF16, tag="xb")
                        nc.vector.tensor_scalar_mul(xb[:], xf[:], S_X)
                        for eb in range(EB):
                            pt = pa_ps.tile([P, P], BF16, tag="tp")
                            nc.tensor.transpose(
                                pt[:], xb[:, eb * P:(eb + 1) * P], ident[:])
                            nc.scalar.copy(
                                xT[:, eb, sb * P:(sb + 1) * P], pt[:])

                with tc.tile_pool(name="pkv", bufs=1) as pkv:
                    AT = pkv.tile([P, EB, S], F8)   # S_A*(x@M)^T
                    VW = pkv.tile([P, SB, E], F8)   # S_VW*(x@NP), [k, f]

                    # ---- Phase B: AT, VW ----------------------------------
                    with tc.tile_pool(name="wm", bufs=1) as wm, \
                         tc.tile_pool(name="pb_ps", bufs=4, space="PSUM") as pb_ps:
                        m_sb = wm.tile([P, EB, E], F8)
                        np_sb = wm.tile([P, EB, E], F8)
                        nc.scalar.dma_start(m_sb[:], m_d[:].rearrange(
                            "p (o n) -> p o n", n=E))
                        nc.scalar.dma_start(np_sb[:], np_d[:].rearrange(
                            "p (o n) -> p o n", n=E))
                        for eb in range(EB):
                            for qc in range(QC):
                                ps = pb_ps.tile([P, NCHUNK], F32, tag="mm")
                                for e2 in range(EB // 2):
                                    nc.tensor.matmul(
                                        ps[:],
                                        m_sb[:, 2 * e2:2 * e2 + 2,
                                             eb * P:(eb + 1) * P],
                                        xT[:, 2 * e2:2 * e2 + 2,
                                           qc * NCHUNK:(qc + 1) * NCHUNK],
                                        start=(e2 == 0), stop=(e2 == EB // 2 - 1),
                                        perf_mode=DR)
                                nc.scalar.activation(
                                    AT[:, eb, qc * NCHUNK:(qc + 1) * NCHUNK],
                                    ps[:], AF.Copy, scale=C_A)
                        for sb in range(SB):
                            for ec in range(E // NCHUNK):
                                ps = pb_ps.tile([P, NCHUNK], F32, tag="mm")
                                for e2 in range(EB // 2):
                                    nc.tensor.matmul(
                                        ps[:],
                                        xT[:, 2 * e2:2 * e2 + 2,
                                           sb * P:(sb + 1) * P],
                                        np_sb[:, 2 * e2:2 * e2 + 2,
                                              ec * NCHUNK:(ec + 1) * NCHUNK],
                                        start=(e2 == 0), stop=(e2 == EB // 2 - 1),
                                        perf_mode=DR)
                                nc.scalar.activation(
                                    VW[:, sb, ec * NCHUNK:(ec + 1) * NCHUNK],
                                    ps[:], AF.Copy, scale=C_VW)

                    # ---- Phase C: attention + proj, LN1 interleaved -------
                    with tc.tile_pool(name="pexp", bufs=2) as pexp, \
                         tc.tile_pool(name="pcw", bufs=1) as pcw, \
                         tc.tile_pool(name="pproj", bufs=2 * QPC) as pproj, \
                         tc.tile_pool(name="lnc", bufs=1) as lnc, \
                         tc.tile_pool(name="pdw", bufs=2) as pdw, \
                         tc.tile_pool(name="pc_ps", bufs=3, space="PSUM") as pc_ps, \
                         tc.tile_pool(name="pp_ps", bufs=2, space="PSUM") as pp_ps, \
                         tc.tile_pool(name="pr_ps", bufs=1, space="PSUM") as pr_ps, \
                         tc.tile_pool(name="pdt_ps", bufs=2, space="PSUM") as pdt_ps:
                        if identity:
                            bo2_b = g1_b = b1_b = None
                        else:
                            bo2_b = lnc.tile([P, E], F32)
                            g1_b = lnc.tile([P, E], F32)
                            b1_b = lnc.tile([P, E], F32)
                            nc.sync.dma_start(bo2_b[:], _bcast_ap(bo2_d[:]))
                            nc.sync.dma_start(g1_b[:], _bcast_ap(g1_d[:]))
                            nc.sync.dma_start(b1_b[:], _bcast_ap(b1_d[:]))
                        hT_r = hT_d[:].rearrange("(o p) s -> p o s", p=P)
                        proj_tiles = {}

                        def d_chain(sb):
                            """residual + LN1 + transpose for one seq block."""
                            xf = pdw.tile([P, E], F32, tag="xres")
                            nc.sync.dma_start(xf[:], x_d[sb * P:(sb + 1) * P, :])
                            hpre = pdw.tile([P, E], F32, tag="hpre")
                            nc.vector.tensor_scalar(hpre[:],
                                                    proj_tiles.pop(sb)[:],
                                                    recip_sb[:, sb:sb + 1],
                                                    None, ALU.mult)
                            if not identity:
                                nc.vector.tensor_add(hpre[:], hpre[:], bo2_b[:])
                            nc.vector.tensor_add(hpre[:], hpre[:], xf[:])
                            _layer_norm_inplace(nc, pdw, hpre, g1_b, b1_b, eps_c)
                            nc.sync.dma_start(h_d[sb * P:(sb + 1) * P, :], hpre[:])
                            hb16 = pdw.tile([P, E], BF16, tag="hb16")
                            nc.vector.tensor_scalar_mul(hb16[:], hpre[:], S_H)
                            hTt = pdw.tile([P, EB, P], F8, tag="hTt")
                            for eb in range(EB):
                                pt = pdt_ps.tile([P, P], BF16, tag="tp2")
                                nc.tensor.transpose(
                                    pt[:], hb16[:, eb * P:(eb + 1) * P], ident[:])
                                nc.scalar.copy(hTt[:, eb, :], pt[:])
                            nc.scalar.dma_start(
                                hT_r[:, :, sb * P:(sb + 1) * P], hTt[:])

                        for qc in range(QC):
                            expS = pexp.tile([P, SB, NCHUNK], F8, tag="expS")
                            for kb in range(SB):
                                ps = pc_ps.tile([P, NCHUNK], F32, tag="s")
                                for e2 in range(EB // 2):
                                    nc.tensor.matmul(
                                        ps[:],
                                        xT[:, 2 * e2:2 * e2 + 2,
                                           kb * P:(kb + 1) * P],
                                        AT[:, 2 * e2:2 * e2 + 2,
                                           qc * NCHUNK:(qc + 1) * NCHUNK],
                                        start=(e2 == 0), stop=(e2 == EB // 2 - 1),
                                        perf_mode=DR)
                                nc.scalar.activation(
                                    expS[:, kb, :], ps[:], AF.Exp,
                                    bias=w_sb[:, kb:kb + 1], scale=C_EXP)
                            acc = [None] * 8
                            for j in range(8):
                                a = pcw.tile([P, NCHUNK], F32, tag=f"acc{j}")
                                nc.vector.tensor_add(a[:], expS[:, j, :],
                                                     expS[:, j + 8, :])
                                acc[j] = a
                            for j in range(4):
                                nc.vector.tensor_add(acc[j][:], acc[j][:],
                                                     acc[j + 4][:])
                            for j in range(2):
                                nc.vector.tensor_add(acc[j][:], acc[j][:],
                                                     acc[j + 2][:])
                            nc.vector.tensor_add(acc[0][:], acc[0][:], acc[1][:])
                            for qs in range(QPC):
                                sb = qc * QPC + qs
                                proj = pproj.tile([P, E], BF16, tag="proj")
                                proj_tiles[sb] = proj
                                for fc in range(E // NCHUNK):
                                    ps = pp_ps.tile([P, NCHUNK], F32, tag="pp")
                                    for k2 in range(SB // 2):
                                        nc.tensor.matmul(
                                            ps[:],
                                            expS[:, 2 * k2:2 * k2 + 2,
                                                 qs * P:(qs + 1) * P],
                                            VW[:, 2 * k2:2 * k2 + 2,
                                               fc * NCHUNK:(fc + 1) * NCHUNK],
                                            start=(k2 == 0),
                                            stop=(k2 == SB // 2 - 1),
                                            perf_mode=DR)
                                    nc.scalar.activation(
                                        proj[:, fc * NCHUNK:(fc + 1) * NCHUNK],
                                        ps[:], AF.Copy, scale=1.0 / S_VW)
                            for qs in range(QPC):
                                sb = qc * QPC + qs
                                pr = pr_ps.tile([P, 1], F32, tag="rs")
                                nc.tensor.matmul(pr[:],
                                                 acc[0][:, qs * P:(qs + 1) * P],
                                                 ones_c[:], start=True, stop=True)
                                nc.vector.reciprocal(recip_sb[:, sb:sb + 1], pr[:])
                            if qc > 0:
                                for qs in range(QPC):
                                    d_chain((qc - 1) * QPC + qs)
                        for qs in range(QPC):
                            d_chain((QC - 1) * QPC + qs)
                # pkv, pbig closed

            # ---- Phase E: FFN + LN2 + out ---------------------------------
            with tc.tile_pool(name="w2r", bufs=1) as w2r, \
                 tc.tile_pool(name="lnc2", bufs=1) as lnc2, \
                 tc.tile_pool(name="pht", bufs=2) as pht, \
                 tc.tile_pool(name="pr1a", bufs=2) as pr1a, \
                 tc.tile_pool(name="pew", bufs=3) as pew, \
                 tc.tile_pool(name="pr1_ps", bufs=3, space="PSUM") as pr1_ps, \
                 tc.tile_pool(name="pf2_ps", bufs=4, space="PSUM") as pf2_ps:
                # W1 resident in SBUF for the whole phase (loaded once, not
                # re-streamed per group: saves 12MB of repeat DMA)
                w1_sb = w2r.tile([P, HB // 4, 4, EB, P], F8)
                for c in range(HB // 4):
                    nc.gpsimd.dma_start(
                        w1_sb[:, c], w1_d[c].rearrange(
                            "p (t o n) -> p t o n", t=4, o=EB))
                w2_sb = w2r.tile([P, HB, E], F8)
                w2_r = w2_d[:].rearrange("p (o n) -> p o n", n=E)
                for hq in range(4):
                    nc.sync.dma_start(
                        w2_sb[:, hq * (HB // 4):(hq + 1) * (HB // 4), :],
                        w2_r[:, hq * (HB // 4):(hq + 1) * (HB // 4), :])
                if identity:
                    bf2_b = g2_b = b2_b = None
                else:
                    bf2_b = lnc2.tile([P, E], F32)
                    g2_b = lnc2.tile([P, E], F32)
                    b2_b = lnc2.tile([P, E], F32)
                    nc.sync.dma_start(bf2_b[:], _bcast_ap(bf2_d[:]))
                    nc.sync.dma_start(g2_b[:], _bcast_ap(g2_d[:]))
                    nc.sync.dma_start(b2_b[:], _bcast_ap(b2_d[:]))
                hT_r = hT_d[:].rearrange("(o p) s -> p o s", p=P)
                QW = 4 * P  # 4 seq blocks per group
                for g in range(S // QW):
                    hts = pht.tile([P, EB, QW], F8, tag="hts")
                    nc.scalar.dma_start(hts[:], hT_r[:, :, g * QW:(g + 1) * QW])
                    r1_all = pr1a.tile([P, HB, QW], F8, tag="r1a")
                    for c in range(HB // 4):
                        for t in range(4):
                            hb = c * 4 + t
                            ps1 = pr1_ps.tile([P, QW], F32, tag="r1")
                            for e2 in range(EB // 2):
                                nc.tensor.matmul(
                                    ps1[:],
                                    w1_sb[:, c, t, 2 * e2:2 * e2 + 2, :],
                                    hts[:, 2 * e2:2 * e2 + 2, :],
                                    start=(e2 == 0), stop=(e2 == EB // 2 - 1),
                                    perf_mode=DR)
                            nc.scalar.activation(r1_all[:, hb, :], ps1[:], AF.Relu,
                                                 bias=bf1_sb[:, hb:hb + 1],
                                                 scale=C_R1)
                    for i in range(QW // P):
                        sb = g * (QW // P) + i
                        t = pew.tile([P, E], F32, tag="ffn")
                        for j in range(E // NCHUNK):
                            ps = pf2_ps.tile([P, NCHUNK], F32, tag="f2")
                            for h2 in range(HB // 2):
                                nc.tensor.matmul(
                                    ps[:],
                                    r1_all[:, 2 * h2:2 * h2 + 2,
                                           i * P:(i + 1) * P],
                                    w2_sb[:, 2 * h2:2 * h2 + 2,
                                          j * NCHUNK:(j + 1) * NCHUNK],
                                    start=(h2 == 0), stop=(h2 == HB // 2 - 1),
                                    perf_mode=DR)
                            nc.vector.tensor_scalar_mul(
                                t[:, j * NCHUNK:(j + 1) * NCHUNK], ps[:], C_F2)
                        hres = pew.tile([P, E], F32, tag="hres")
                        nc.sync.dma_start(hres[:], h_d[sb * P:(sb + 1) * P, :])
                        if not identity:
                            nc.vector.tensor_add(hres[:], hres[:], bf2_b[:])
                        nc.vector.tensor_add(t[:], t[:], hres[:])
                        _layer_norm_inplace(nc, pew, t, g2_b, b2_b, eps_c)
                        nc.sync.dma_start(out_d[sb * P:(sb + 1) * P, :], t[:])

    nc.compile()
    return nc


def _get_nc(identity):
    if identity not in _CACHED_NC:
        _CACHED_NC[identity] = build_nc(identity)
    return _CACHED_NC[identity]


def kernel(**inputs):
    x = np.ascontiguousarray(np.asarray(inputs["x"], dtype=np.float32))
    B = x.shape[0]
    assert x.shape == (8, S, E), x.shape

    def q8(a, s):
        v = np.clip(np.asarray(a, np.float64) * s, -240.0, 240.0)
        return np.ascontiguousarray(v.astype(np.float32)
                                    .astype(ml_dtypes.float8_e4m3))

    def f32(a):
        return np.ascontiguousarray(np.asarray(a, dtype=np.float32))

    Wq = np.asarray(inputs["Wq"], np.float32)
    Wk = np.asarray(inputs["Wk"], np.float32)
    Wv = np.asarray(inputs["Wv"], np.float32)
    Wo = np.asarray(inputs["Wo"], np.float32)
    bq = np.asarray(inputs["bq"], np.float32)
    bk = np.asarray(inputs["bk"], np.float32)
    bv = np.asarray(inputs["bv"], np.float32)
    bo = np.asarray(inputs["bo"], np.float32)
    W1 = np.asarray(inputs["W1"], np.float32)
    W2 = np.asarray(inputs["W2"], np.float32)
    scale = np.float32(SCALE)

    M = Wq @ Wk.T
    NP_ = Wv @ Wo
    # shuffles: row p of Ms holds M[o*128+p, :] blocks concatenated over o
    Ms = q8(M.reshape(EB, P, E).transpose(1, 0, 2).reshape(P, EB * E), S_M)
    NPs = q8(NP_.reshape(EB, P, E).transpose(1, 0, 2).reshape(P, EB * E), S_NP)
    # W1s[c, p, t*E + ei*128 + j] = W1[ei*128+p, (4c+t)*128+j]
    W1s = q8(W1.reshape(EB, P, HB // 4, 4, P)
             .transpose(2, 1, 3, 0, 4).reshape(HB // 4, P, 4 * E), S_W1)
    W2s = q8(W2.reshape(HB, P, E).transpose(1, 0, 2).reshape(P, HB * E), S_W2)

    shared = {
        "Ms": Ms, "NPs": NPs, "W1s": W1s, "W2s": W2s,
        "bo2": f32(bo + bv @ Wo),
        "bf1": f32(np.asarray(inputs["bf1"], np.float32) * S_R),
        "bf2": f32(inputs["bf2"]),
        "g1": f32(inputs["g1"]), "b1": f32(inputs["b1"]),
        "g2": f32(inputs["g2"]), "b2": f32(inputs["b2"]),
    }
    vq = Wk @ bq
    cq = float(bq @ bk)
    lse = np.float32(np.log(S_E))
    in_maps = [
        {"x": x[c], "wrow": f32(scale * (x[c] @ vq) + scale * cq + lse),
         **shared}
        for c in range(B)
    ]

    identity = bool(
        np.all(np.asarray(inputs["g1"], np.float32) == 1.0)
        and np.all(np.asarray(inputs["b1"], np.float32) == 0.0)
        and np.all(np.asarray(inputs["g2"], np.float32) == 1.0)
        and np.all(np.asarray(inputs["b2"], np.float32) == 0.0)
        and np.all(shared["bo2"] == 0.0)
        and np.all(shared["bf2"] == 0.0)
    )
    nc = _get_nc(identity)
    trace = bool(int(os.environ.get("BERT_TRACE", "0")))
    res = run_bass_kernel_spmd(nc, in_maps, core_ids=list(range(B)), trace=trace)
    if trace and res.exec_time_ns is not None:
        print(f"HW exec time: {res.exec_time_ns} ns")
        kernel.last_exec_time_ns = res.exec_time_ns
        kernel.last_trace = res.instructions_and_trace
    return np.stack([res.results[c]["out"] for c in range(B)]).astype(np.float32)


# revision 28
# speedup vs baseline: 1.0264x; 1.0264x over previous
"""BERT encoder block on 8 Trainium2 NeuronCores.

Strategy: pure data parallelism — batch 8 is split one batch element per core
(no collectives). Each core runs the full encoder block on its [2048, 1024]
slice. All six big matmuls run in fp8 (TRN e4m3) DoubleRow mode (2 fp8
weights per PE cell -> 2x contraction per pass); accumulation and the
residual/LN stream are fp32.

Algebraic folds done on the host (softmax row-invariance absorbs the
query-side bias term):
  M   = Wq @ Wk^T          -> scores = x M x^T  (one fused tensor A = x@M)
  NP  = Wv @ Wo            -> attn_out @ Wo = softmax(S) @ (x@NP) + bo2
  bo2 = bo + bv @ Wo
  wrow = scale*(x @ (Wk@bq) + bq.bk) + ln(S_E)   (key-side score bias)

fp8 scaling: every fp8 tensor T is stored as s_T * T with a power-of-2
per-tensor scale chosen so absmax stays well under TRN e4m3's +-240 (and
above the 2^-6 subnormal threshold for typical values). All descales fold
into existing activation scale/bias parameters:
  xT   = S_X  * x^T            (scaled during the f32->bf16 pre-transpose copy)
  Ms   = S_M  * M, NPs = S_NP * NP, W1s = S_W1 * W1, W2s = S_W2 * W2  (host)
  AT   = S_A  * (x@M)^T        (PSUM copy scale = S_A/(S_M*S_X))
  VW   = S_VW * (x@NP)         (PSUM copy scale = S_VW/(S_X*S_NP))
  expS = S_E  * exp(..)        (ln S_E folded into wrow bias on host)
  proj = S_E * num             (PV psum copy scale 1/S_VW; the softmax
  normalization 1/(S_E*D) is applied per-row in d_chain on VectorE, which
  decouples the PV matmul stream from the denominator reduction)
  hT   = S_H  * h^T            (scaled during the f32->bf16 pre-transpose copy)
  r1   = S_R  * relu(..)       (relu scale = S_R/(S_W1*S_H), bias = S_R*bf1)
  FFN2 out descale = 1/(S_R*S_W2) via one tensor_scalar_mul.
The fp32 residual/LN stream is never scaled.

Attention runs in transposed score layout S^T[k,q]: softmax denominators are
ones-matmuls and proj = P @ (x@NP) lands directly in [q, f] layout. The
residual+LN1+h-transpose work is interleaved into the attention loop with a
one-chunk lag so TensorE never drains. The transposed LN1 output hT stays
resident in SBUF (no DRAM round-trip) and the FFN weights are prefetched on
the gpsimd DMA queue during phases A..C, so Phase E starts without stalls.
Weights are pre-shuffled on the host so every big DMA is one contiguous
segment per partition.

Self-contained: hardcodes shapes from the problem spec.
"""
import os

import numpy as np
import ml_dtypes

import concourse.bacc as bacc
import concourse.bass as bass
import concourse.tile as tile
import concourse.mybir as mybir
from concourse.bass_utils import run_bass_kernel_spmd
from concourse.masks import make_identity

P = 128
S = 2048          # sequence length per core
E = 1024          # embed
F = 4096          # ffn hidden
SB = S // P       # 16 seq blocks
EB = E // P       # 8 embed blocks
HB = F // P       # 32 ffn blocks
NCHUNK = 512
QC = S // NCHUNK  # 4 q chunks
QPC = NCHUNK // P  # 4 seq blocks per chunk
LN_EPS = 1e-5
SCALE = 1.0 / np.sqrt(np.float32(E))

# fp8 per-tensor scales (powers of 2; absmaxes measured on the fixed input
# distribution with ~2x safety margin under TRN e4m3's +-240)
S_X = 32.0    # x absmax ~5.4   -> 173
S_M = 2048.0  # M absmax ~.058  -> 118
S_NP = 2048.0  # NP absmax ~.053 -> 107
S_A = 32.0    # A absmax ~1.9   -> 62
S_VW = 64.0   # VW absmax ~1.8  -> 115
S_E = 16.0    # exp absmax ~7.5 -> 119
S_H = 32.0    # h absmax ~4.9   -> 158
S_W1 = 4096.0  # W1 absmax 1/32 -> 128
S_R = 16.0    # r1 absmax ~3.2  -> 50
S_W2 = 8192.0  # W2 absmax 1/64 -> 128

C_A = S_A / (S_M * S_X)        # AT copy scale
C_VW = S_VW / (S_X * S_NP)     # VW copy scale
C_EXP = float(SCALE) / (S_X * S_A)  # exp activation scale
C_R1 = S_R / (S_W1 * S_H)      # relu activation scale
C_F2 = 1.0 / (S_R * S_W2)      # FFN2 descale

F32 = mybir.dt.float32
BF16 = mybir.dt.bfloat16
F8 = mybir.dt.float8e4
AF = mybir.ActivationFunctionType
ALU = mybir.AluOpType
DR = mybir.MatmulPerfMode.DoubleRow

_CACHED_NC = {}


def _bcast_ap(ap, parts=P):
    """DRAM row-vector -> [parts, n] partition-broadcast access pattern."""
    return bass.AP(tensor=ap.tensor, offset=ap.offset,
                   ap=[[0, parts]] + [list(d) for d in ap.ap])


def _layer_norm_inplace(nc, work, src, gamma, beta, eps_c):
    """LN over free dim of src [P, E] fp32, in place.

    gamma/beta None means identity (fold for the common g=1, b=0 case).
    """
    stats = work.tile([P, 2, 6], F32, tag="ln_stats")
    nc.vector.bn_stats(stats[:, 0, :], src[:, 0:512])
    nc.vector.bn_stats(stats[:, 1, :], src[:, 512:1024])
    mv = work.tile([P, 2], F32, tag="ln_mv")
    nc.vector.bn_aggr(mv[:], stats[:])
    std = work.tile([P, 1], F32, tag="ln_std")
    nc.scalar.activation(std[:], mv[:, 1:2], AF.Sqrt, bias=eps_c[:], scale=1.0)
    rstd = work.tile([P, 1], F32, tag="ln_rstd")
    nc.vector.reciprocal(rstd[:], std[:])
    nc.vector.tensor_scalar(src[:], src[:], mv[:, 0:1], rstd[:],
                            ALU.subtract, ALU.mult)
    if gamma is not None:
        nc.vector.tensor_mul(src[:], src[:], gamma[:])
    if beta is not None:
        nc.vector.tensor_add(src[:], src[:], beta[:])


def build_nc(identity=False):
    """identity=True folds away LN gamma/beta and the bo2/bf2 bias adds,
    valid when g1=g2=1, b1=b2=bo2=bf2=0 (checked on the host)."""
    nc = bacc.Bacc(None, target_bir_lowering=False, debug=False)

    x_d = nc.dram_tensor("x", [S, E], F32, kind="ExternalInput")
    # host-preshuffled: row p holds M[o*128+p, :] for o in 0..7, concatenated
    m_d = nc.dram_tensor("Ms", [P, EB * E], F8, kind="ExternalInput")
    np_d = nc.dram_tensor("NPs", [P, EB * E], F8, kind="ExternalInput")
    # W1s[c, p, t*E + ei*128 + j] = W1[ei*128+p, (4c+t)*128+j]
    w1_d = nc.dram_tensor("W1s", [HB // 4, P, 4 * E], F8, kind="ExternalInput")
    # W2s[p, hb*E + n] = W2[hb*128+p, n]
    w2_d = nc.dram_tensor("W2s", [P, HB * E], F8, kind="ExternalInput")
    bo2_d = nc.dram_tensor("bo2", [E], F32, kind="ExternalInput")
    wrow_d = nc.dram_tensor("wrow", [S], F32, kind="ExternalInput")
    bf1_d = nc.dram_tensor("bf1", [F], F32, kind="ExternalInput")
    bf2_d = nc.dram_tensor("bf2", [E], F32, kind="ExternalInput")
    g1_d = nc.dram_tensor("g1", [E], F32, kind="ExternalInput")
    b1_d = nc.dram_tensor("b1", [E], F32, kind="ExternalInput")
    g2_d = nc.dram_tensor("g2", [E], F32, kind="ExternalInput")
    b2_d = nc.dram_tensor("b2", [E], F32, kind="ExternalInput")
    out_d = nc.dram_tensor("out", [S, E], F32, kind="ExternalOutput")
    h_d = nc.dram_tensor("h_scratch", [S, E], F32)   # LN1 output spill

    with tile.TileContext(nc, pool_alloc_mode="queue") as tc:
        with tc.tile_pool(name="const", bufs=1) as const:
            ident = const.tile([P, P], BF16)
            make_identity(nc, ident)
            ones_c = const.tile([P, 1], F32)
            nc.vector.memset(ones_c[:], 1.0)
            eps_c = const.tile([P, 1], F32)
            nc.vector.memset(eps_c[:], LN_EPS)
            # keep these strided gathers off the sync queue so the first
            # x-tile DMA isn't stuck behind them
            bf1_sb = const.tile([P, HB], F32)
            nc.gpsimd.dma_start(bf1_sb[:], bf1_d[:].rearrange("(o p) -> p o", p=P))
            recip_sb = const.tile([P, SB], F32)
            w_sb = const.tile([P, SB], F32)
            nc.gpsimd.dma_start(w_sb[:], wrow_d[:].rearrange("(o p) -> p o", p=P))

            with tc.tile_pool(name="pres", bufs=1) as pres:
                # resident through phase E: transposed LN1 output (built in
                # phase C, consumed in phase E — never leaves SBUF) and the
                # FFN weights, prefetched on the gpsimd queue from t=0.
                hT_sb = pres.tile([P, EB, S], F8)   # S_H * h^T
                w1_sb = pres.tile([P, HB // 4, 4, EB, P], F8)
                for c in range(HB // 4):
                    nc.gpsimd.dma_start(
                        w1_sb[:, c], w1_d[c].rearrange(
                            "p (t o n) -> p t o n", t=4, o=EB))

                with tc.tile_pool(name="pbig", bufs=1) as pbig:
                    xT = pbig.tile([P, EB, S], F8)  # S_X*x^T

                    # ---- Phase A: load x, scale, transpose to xT ----------
                    with tc.tile_pool(name="pa", bufs=3) as pa, \
                         tc.tile_pool(name="pa_ps", bufs=4, space="PSUM") as pa_ps:
                        for sb in range(SB):
                            xf = pa.tile([P, E], F32, tag="xf")
                            nc.sync.dma_start(xf[:], x_d[sb * P:(sb + 1) * P, :])
                            xb = pa.tile([P, E], BF16, tag="xb")
                            nc.vector.tensor_scalar_mul(xb[:], xf[:], S_X)
                            for eb in range(EB):
                                pt = pa_ps.tile([P, P], BF16, tag="tp")
                                nc.tensor.transpose(
                                    pt[:], xb[:, eb * P:(eb + 1) * P], ident[:])
                                nc.scalar.copy(
                                    xT[:, eb, sb * P:(sb + 1) * P], pt[:])

                    with tc.tile_pool(name="pkv", bufs=1) as pkv:
                        AT = pkv.tile([P, EB, S], F8)   # S_A*(x@M)^T
                        VW = pkv.tile([P, SB, E], F8)   # S_VW*(x@NP), [k, f]

                        # ---- Phase B: AT, VW ------------------------------
                        with tc.tile_pool(name="wm", bufs=1) as wm, \
                             tc.tile_pool(name="pb_ps", bufs=4,
                                          space="PSUM") as pb_ps:
                            m_sb = wm.tile([P, EB, E], F8)
                            np_sb = wm.tile([P, EB, E], F8)
                            nc.scalar.dma_start(m_sb[:], m_d[:].rearrange(
                                "p (o n) -> p o n", n=E))
                            nc.scalar.dma_start(np_sb[:], np_d[:].rearrange(
                                "p (o n) -> p o n", n=E))
                            for eb in range(EB):
                                for qc in range(QC):
                                    ps = pb_ps.tile([P, NCHUNK], F32, tag="mm")
                                    for e2 in range(EB // 2):
                                        nc.tensor.matmul(
                                            ps[:],
                                            m_sb[:, 2 * e2:2 * e2 + 2,
                                                 eb * P:(eb + 1) * P],
                                            xT[:, 2 * e2:2 * e2 + 2,
                                               qc * NCHUNK:(qc + 1) * NCHUNK],
                                            start=(e2 == 0),
                                            stop=(e2 == EB // 2 - 1),
                                            perf_mode=DR)
                                    nc.scalar.activation(
                                        AT[:, eb, qc * NCHUNK:(qc + 1) * NCHUNK],
                                        ps[:], AF.Copy, scale=C_A)
                            for sb in range(SB):
                                for ec in range(E // NCHUNK):
                                    ps = pb_ps.tile([P, NCHUNK], F32, tag="mm")
                                    for e2 in range(EB // 2):
                                        nc.tensor.matmul(
                                            ps[:],
                                            xT[:, 2 * e2:2 * e2 + 2,
                                               sb * P:(sb + 1) * P],
                                            np_sb[:, 2 * e2:2 * e2 + 2,
                                                  ec * NCHUNK:(ec + 1) * NCHUNK],
                                            start=(e2 == 0),
                                            stop=(e2 == EB // 2 - 1),
                                            perf_mode=DR)
                                    nc.scalar.activation(
                                        VW[:, sb, ec * NCHUNK:(ec + 1) * NCHUNK],
                                        ps[:], AF.Copy, scale=C_VW)

                        # ---- Phase C: attention + proj, LN1 interleaved ---
                        with tc.tile_pool(name="pexp", bufs=2) as pexp, \
                             tc.tile_pool(name="pcw", bufs=1) as pcw, \
                             tc.tile_pool(name="pproj", bufs=2 * QPC) as pproj, \
                             tc.tile_pool(name="lnc", bufs=1) as lnc, \
                             tc.tile_pool(name="pdw", bufs=2) as pdw, \
                             tc.tile_pool(name="pc_ps", bufs=3,
                                          space="PSUM") as pc_ps, \
                             tc.tile_pool(name="pp_ps", bufs=2,
                                          space="PSUM") as pp_ps, \
                             tc.tile_pool(name="pr_ps", bufs=1,
                                          space="PSUM") as pr_ps, \
                             tc.tile_pool(name="pdt_ps", bufs=2,
                                          space="PSUM") as pdt_ps:
                            if identity:
                                bo2_b = g1_b = b1_b = None
                            else:
                                bo2_b = lnc.tile([P, E], F32)
                                g1_b = lnc.tile([P, E], F32)
                                b1_b = lnc.tile([P, E], F32)
                                nc.sync.dma_start(bo2_b[:], _bcast_ap(bo2_d[:]))
                                nc.sync.dma_start(g1_b[:], _bcast_ap(g1_d[:]))
                                nc.sync.dma_start(b1_b[:], _bcast_ap(b1_d[:]))
                            proj_tiles = {}

                            def d_chain(sb):
                                """residual + LN1 + transpose, one seq block."""
                                xf = pdw.tile([P, E], F32, tag="xres")
                                nc.sync.dma_start(xf[:],
                                                  x_d[sb * P:(sb + 1) * P, :])
                                hpre = pdw.tile([P, E], F32, tag="hpre")
                                nc.vector.tensor_scalar(hpre[:],
                                                        proj_tiles.pop(sb)[:],
                                                        recip_sb[:, sb:sb + 1],
                                                        None, ALU.mult)
                                if not identity:
                                    nc.vector.tensor_add(hpre[:], hpre[:],
                                                         bo2_b[:])
                                nc.vector.tensor_add(hpre[:], hpre[:], xf[:])
                                _layer_norm_inplace(nc, pdw, hpre, g1_b, b1_b,
                                                    eps_c)
                                nc.sync.dma_start(h_d[sb * P:(sb + 1) * P, :],
                                                  hpre[:])
                                hb16 = pdw.tile([P, E], BF16, tag="hb16")
                                nc.vector.tensor_scalar_mul(hb16[:], hpre[:], S_H)
                                for eb in range(EB):
                                    pt = pdt_ps.tile([P, P], BF16, tag="tp2")
                                    nc.tensor.transpose(
                                        pt[:], hb16[:, eb * P:(eb + 1) * P],
                                        ident[:])
                                    nc.scalar.copy(
                                        hT_sb[:, eb, sb * P:(sb + 1) * P], pt[:])

                            for qc in range(QC):
                                expS = pexp.tile([P, SB, NCHUNK], F8, tag="expS")
                                for kb in range(SB):
                                    ps = pc_ps.tile([P, NCHUNK], F32, tag="s")
                                    for e2 in range(EB // 2):
                                        nc.tensor.matmul(
                                            ps[:],
                                            xT[:, 2 * e2:2 * e2 + 2,
                                               kb * P:(kb + 1) * P],
                                            AT[:, 2 * e2:2 * e2 + 2,
                                               qc * NCHUNK:(qc + 1) * NCHUNK],
                                            start=(e2 == 0),
                                            stop=(e2 == EB // 2 - 1),
                                            perf_mode=DR)
                                    nc.scalar.activation(
                                        expS[:, kb, :], ps[:], AF.Exp,
                                        bias=w_sb[:, kb:kb + 1], scale=C_EXP)
                                acc = [None] * 8
                                for j in range(8):
                                    a = pcw.tile([P, NCHUNK], F32, tag=f"acc{j}")
                                    nc.vector.tensor_add(a[:], expS[:, j, :],
                                                         expS[:, j + 8, :])
                                    acc[j] = a
                                for j in range(4):
                                    nc.vector.tensor_add(acc[j][:], acc[j][:],
                                                         acc[j + 4][:])
                                for j in range(2):
                                    nc.vector.tensor_add(acc[j][:], acc[j][:],
                                                         acc[j + 2][:])
                                nc.vector.tensor_add(acc[0][:], acc[0][:],
                                                     acc[1][:])
                                for qs in range(QPC):
                                    sb = qc * QPC + qs
                                    proj = pproj.tile([P, E], BF16, tag="proj")
                                    proj_tiles[sb] = proj
                                    for fc in range(E // NCHUNK):
                                        ps = pp_ps.tile([P, NCHUNK], F32,
                                                        tag="pp")
                                        for k2 in range(SB // 2):
                                            nc.tensor.matmul(
                                                ps[:],
                                                expS[:, 2 * k2:2 * k2 + 2,
                                                     qs * P:(qs + 1) * P],
                                                VW[:, 2 * k2:2 * k2 + 2,
                                                   fc * NCHUNK:(fc + 1) * NCHUNK],
                                                start=(k2 == 0),
                                                stop=(k2 == SB // 2 - 1),
                                                perf_mode=DR)
                                        nc.scalar.activation(
                                            proj[:, fc * NCHUNK:(fc + 1) * NCHUNK],
                                            ps[:], AF.Copy, scale=1.0 / S_VW)
                                for qs in range(QPC):
                                    sb = qc * QPC + qs
                                    pr = pr_ps.tile([P, 1], F32, tag="rs")
                                    nc.tensor.matmul(
                                        pr[:], acc[0][:, qs * P:(qs + 1) * P],
                                        ones_c[:], start=True, stop=True)
                                    nc.vector.reciprocal(recip_sb[:, sb:sb + 1],
                                                         pr[:])
                                if qc > 0:
                                    for qs in range(QPC):
                                        d_chain((qc - 1) * QPC + qs)
                            for qs in range(QPC):
                                d_chain((QC - 1) * QPC + qs)
                    # pkv, pbig closed

                # ---- Phase E: FFN + LN2 + out -----------------------------
                with tc.tile_pool(name="w2r", bufs=1) as w2r, \
                     tc.tile_pool(name="lnc2", bufs=1) as lnc2, \
                     tc.tile_pool(name="pr1a", bufs=2) as pr1a, \
                     tc.tile_pool(name="pew", bufs=3) as pew, \
                     tc.tile_pool(name="pr1_ps", bufs=3, space="PSUM") as pr1_ps, \
                     tc.tile_pool(name="pf2_ps", bufs=4, space="PSUM") as pf2_ps:
                    w2_sb = w2r.tile([P, HB, E], F8)
                    w2_r = w2_d[:].rearrange("p (o n) -> p o n", n=E)
                    for hq in range(4):
                        nc.gpsimd.dma_start(
                            w2_sb[:, hq * (HB // 4):(hq + 1) * (HB // 4), :],
                            w2_r[:, hq * (HB // 4):(hq + 1) * (HB // 4), :])
                    if identity:
                        bf2_b = g2_b = b2_b = None
                    else:
                        bf2_b = lnc2.tile([P, E], F32)
                        g2_b = lnc2.tile([P, E], F32)
                        b2_b = lnc2.tile([P, E], F32)
                        nc.sync.dma_start(bf2_b[:], _bcast_ap(bf2_d[:]))
                        nc.sync.dma_start(g2_b[:], _bcast_ap(g2_d[:]))
                        nc.sync.dma_start(b2_b[:], _bcast_ap(b2_d[:]))
                    QW = 4 * P  # 4 seq blocks per group
                    for g in range(S // QW):
                        r1_all = pr1a.tile([P, HB, QW], F8, tag="r1a")
                        for c in range(HB // 4):
                            for t in range(4):
                                hb = c * 4 + t
                                ps1 = pr1_ps.tile([P, QW], F32, tag="r1")
                                for e2 in range(EB // 2):
                                    nc.tensor.matmul(
                                        ps1[:],
                                        w1_sb[:, c, t, 2 * e2:2 * e2 + 2, :],
                                        hT_sb[:, 2 * e2:2 * e2 + 2,
                                              g * QW:(g + 1) * QW],
                                        start=(e2 == 0),
                                        stop=(e2 == EB // 2 - 1),
                                        perf_mode=DR)
                                nc.scalar.activation(r1_all[:, hb, :], ps1[:],
                                                     AF.Relu,
                                                     bias=bf1_sb[:, hb:hb + 1],
                                                     scale=C_R1)
                        for i in range(QW // P):
                            sb = g * (QW // P) + i
                            t = pew.tile([P, E], F32, tag="ffn")
                            for j in range(E // NCHUNK):
                                ps = pf2_ps.tile([P, NCHUNK], F32, tag="f2")
                                for h2 in range(HB // 2):
                                    nc.tensor.matmul(
                                        ps[:],
                                        r1_all[:, 2 * h2:2 * h2 + 2,
                                               i * P:(i + 1) * P],
                                        w2_sb[:, 2 * h2:2 * h2 + 2,
                                              j * NCHUNK:(j + 1) * NCHUNK],
                                        start=(h2 == 0),
                                        stop=(h2 == HB // 2 - 1),
                                        perf_mode=DR)
                                nc.vector.tensor_scalar_mul(
                                    t[:, j * NCHUNK:(j + 1) * NCHUNK], ps[:],
                                    C_F2)
                            hres = pew.tile([P, E], F32, tag="hres")
                            nc.sync.dma_start(hres[:],
                                              h_d[sb * P:(sb + 1) * P, :])
                            if not identity:
                                nc.vector.tensor_add(hres[:], hres[:], bf2_b[:])
                            nc.vector.tensor_add(t[:], t[:], hres[:])
                            _layer_norm_inplace(nc, pew, t, g2_b, b2_b, eps_c)
                            nc.sync.dma_start(out_d[sb * P:(sb + 1) * P, :],
                                              t[:])

    nc.compile()
    return nc


def _get_nc(identity):
    if identity not in _CACHED_NC:
        _CACHED_NC[identity] = build_nc(identity)
    return _CACHED_NC[identity]


def kernel(**inputs):
    x = np.ascontiguousarray(np.asarray(inputs["x"], dtype=np.float32))
    B = x.shape[0]
    assert x.shape == (8, S, E), x.shape

    def q8(a, s):
        v = np.clip(np.asarray(a, np.float64) * s, -240.0, 240.0)
        return np.ascontiguousarray(v.astype(np.float32)
                                    .astype(ml_dtypes.float8_e4m3))

    def f32(a):
        return np.ascontiguousarray(np.asarray(a, dtype=np.float32))

    Wq = np.asarray(inputs["Wq"], np.float32)
    Wk = np.asarray(inputs["Wk"], np.float32)
    Wv = np.asarray(inputs["Wv"], np.float32)
    Wo = np.asarray(inputs["Wo"], np.float32)
    bq = np.asarray(inputs["bq"], np.float32)
    bk = np.asarray(inputs["bk"], np.float32)
    bv = np.asarray(inputs["bv"], np.float32)
    bo = np.asarray(inputs["bo"], np.float32)
    W1 = np.asarray(inputs["W1"], np.float32)
    W2 = np.asarray(inputs["W2"], np.float32)
    scale = np.float32(SCALE)

    M = Wq @ Wk.T
    NP_ = Wv @ Wo
    # shuffles: row p of Ms holds M[o*128+p, :] blocks concatenated over o
    Ms = q8(M.reshape(EB, P, E).transpose(1, 0, 2).reshape(P, EB * E), S_M)
    NPs = q8(NP_.reshape(EB, P, E).transpose(1, 0, 2).reshape(P, EB * E), S_NP)
    # W1s[c, p, t*E + ei*128 + j] = W1[ei*128+p, (4c+t)*128+j]
    W1s = q8(W1.reshape(EB, P, HB // 4, 4, P)
             .transpose(2, 1, 3, 0, 4).reshape(HB // 4, P, 4 * E), S_W1)
    W2s = q8(W2.reshape(HB, P, E).transpose(1, 0, 2).reshape(P, HB * E), S_W2)

    shared = {
        "Ms": Ms, "NPs": NPs, "W1s": W1s, "W2s": W2s,
        "bo2": f32(bo + bv @ Wo),
        "bf1": f32(np.asarray(inputs["bf1"], np.float32) * S_R),
        "bf2": f32(inputs["bf2"]),
        "g1": f32(inputs["g1"]), "b1": f32(inputs["b1"]),
        "g2": f32(inputs["g2"]), "b2": f32(inputs["b2"]),
    }
    vq = Wk @ bq
    cq = float(bq @ bk)
    lse = np.float32(np.log(S_E))
    in_maps = [
        {"x": x[c], "wrow": f32(scale * (x[c] @ vq) + scale * cq + lse),
         **shared}
        for c in range(B)
    ]

    identity = bool(
        np.all(np.asarray(inputs["g1"], np.float32) == 1.0)
        and np.all(np.asarray(inputs["b1"], np.float32) == 0.0)
        and np.all(np.asarray(inputs["g2"], np.float32) == 1.0)
        and np.all(np.asarray(inputs["b2"], np.float32) == 0.0)
        and np.all(shared["bo2"] == 0.0)
        and np.all(shared["bf2"] == 0.0)
    )
    nc = _get_nc(identity)
    trace = bool(int(os.environ.get("BERT_TRACE", "0")))
    res = run_bass_kernel_spmd(nc, in_maps, core_ids=list(range(B)), trace=trace)
    if trace and res.exec_time_ns is not None:
        print(f"HW exec time: {res.exec_time_ns} ns")
        kernel.last_exec_time_ns = res.exec_time_ns
        kernel.last_trace = res.instructions_and_trace
    return np.stack([res.results[c]["out"] for c in range(B)]).astype(np.float32)


# revision 29
# speedup vs baseline: 1.0328x; 1.0063x over previous
"""BERT encoder block on 8 Trainium2 NeuronCores.

Strategy: pure data parallelism — batch 8 is split one batch element per core
(no collectives). Each core runs the full encoder block on its [2048, 1024]
slice. All six big matmuls run in fp8 (TRN e4m3) DoubleRow mode (2 fp8
weights per PE cell -> 2x contraction per pass); accumulation and the
residual/LN stream are fp32.

Algebraic folds done on the host (softmax row-invariance absorbs the
query-side bias term):
  M   = Wq @ Wk^T          -> scores = x M x^T  (one fused tensor A = x@M)
  NP  = Wv @ Wo            -> attn_out @ Wo = softmax(S) @ (x@NP) + bo2
  bo2 = bo + bv @ Wo
  wrow = scale*(x @ (Wk@bq) + bq.bk) + ln(S_E)   (key-side score bias)

fp8 scaling: every fp8 tensor T is stored as s_T * T with a power-of-2
per-tensor scale chosen so absmax stays well under TRN e4m3's +-240 (and
above the 2^-6 subnormal threshold for typical values). All descales fold
into existing activation scale/bias parameters:
  xT   = S_X  * x^T            (scaled during the f32->bf16 pre-transpose copy)
  Ms   = S_M  * M, NPs = S_NP * NP, W1s = S_W1 * W1, W2s = S_W2 * W2  (host)
  AT   = S_A  * (x@M)^T        (PSUM copy scale = S_A/(S_M*S_X))
  VW   = S_VW * (x@NP)         (PSUM copy scale = S_VW/(S_X*S_NP))
  expS = S_E  * exp(..)        (ln S_E folded into wrow bias on host)
  proj = S_E * num             (PV psum copy scale 1/S_VW; the softmax
  normalization 1/(S_E*D) is applied per-row in d_chain on VectorE, which
  decouples the PV matmul stream from the denominator reduction)
  hT   = S_H  * h^T            (scaled during the f32->bf16 pre-transpose copy)
  r1   = S_R  * relu(..)       (relu scale = S_R/(S_W1*S_H), bias = S_R*bf1)
  FFN2 out descale = 1/(S_R*S_W2) via one tensor_scalar_mul.
The fp32 residual/LN stream is never scaled.

Attention runs in transposed score layout S^T[k,q]: softmax denominators are
ones-matmuls and proj = P @ (x@NP) lands directly in [q, f] layout. The
residual+LN1+h-transpose work is interleaved into the attention loop with a
one-chunk lag so TensorE never drains. The transposed LN1 output hT stays
resident in SBUF (no DRAM round-trip) and the FFN weights are prefetched on
the gpsimd DMA queue during phases A..C, so Phase E starts without stalls.
Weights are pre-shuffled on the host so every big DMA is one contiguous
segment per partition.

Self-contained: hardcodes shapes from the problem spec.
"""
import os

import numpy as np
import ml_dtypes

import concourse.bacc as bacc
import concourse.bass as bass
import concourse.tile as tile
import concourse.mybir as mybir
from concourse.bass_utils import run_bass_kernel_spmd
from concourse.masks import make_identity

P = 128
S = 2048          # sequence length per core
E = 1024          # embed
F = 4096          # ffn hidden
SB = S // P       # 16 seq blocks
EB = E // P       # 8 embed blocks
HB = F // P       # 32 ffn blocks
NCHUNK = 512
QC = S // NCHUNK  # 4 q chunks
QPC = NCHUNK // P  # 4 seq blocks per chunk
LN_EPS = 1e-5
SCALE = 1.0 / np.sqrt(np.float32(E))

# fp8 per-tensor scales (powers of 2; absmaxes measured on the fixed input
# distribution with ~2x safety margin under TRN e4m3's +-240)
S_X = 32.0    # x absmax ~5.4   -> 173
S_M = 2048.0  # M absmax ~.058  -> 118
S_NP = 2048.0  # NP absmax ~.053 -> 107
S_A = 32.0    # A absmax ~1.9   -> 62
S_VW = 64.0   # VW absmax ~1.8  -> 115
S_E = 16.0    # exp absmax ~7.5 -> 119
S_H = 32.0    # h absmax ~4.9   -> 158
S_W1 = 4096.0  # W1 absmax 1/32 -> 128
S_R = 16.0    # r1 absmax ~3.2  -> 50
S_W2 = 8192.0  # W2 absmax 1/64 -> 128

C_A = S_A / (S_M * S_X)        # AT copy scale
C_VW = S_VW / (S_X * S_NP)     # VW copy scale
C_EXP = float(SCALE) / (S_X * S_A)  # exp activation scale
C_R1 = S_R / (S_W1 * S_H)      # relu activation scale
C_F2 = 1.0 / (S_R * S_W2)      # FFN2 descale

F32 = mybir.dt.float32
BF16 = mybir.dt.bfloat16
F8 = mybir.dt.float8e4
AF = mybir.ActivationFunctionType
ALU = mybir.AluOpType
DR = mybir.MatmulPerfMode.DoubleRow

_CACHED_NC = {}


def _bcast_ap(ap, parts=P):
    """DRAM row-vector -> [parts, n] partition-broadcast access pattern."""
    return bass.AP(tensor=ap.tensor, offset=ap.offset,
                   ap=[[0, parts]] + [list(d) for d in ap.ap])


def _layer_norm_inplace(nc, work, src, gamma, beta, eps_c):
    """LN over free dim of src [P, E] fp32, in place.

    gamma/beta None means identity (fold for the common g=1, b=0 case).
    """
    stats = work.tile([P, 2, 6], F32, tag="ln_stats")
    nc.vector.bn_stats(stats[:, 0, :], src[:, 0:512])
    nc.vector.bn_stats(stats[:, 1, :], src[:, 512:1024])
    mv = work.tile([P, 2], F32, tag="ln_mv")
    nc.vector.bn_aggr(mv[:], stats[:])
    std = work.tile([P, 1], F32, tag="ln_std")
    nc.scalar.activation(std[:], mv[:, 1:2], AF.Sqrt, bias=eps_c[:], scale=1.0)
    rstd = work.tile([P, 1], F32, tag="ln_rstd")
    nc.vector.reciprocal(rstd[:], std[:])
    nc.vector.tensor_scalar(src[:], src[:], mv[:, 0:1], rstd[:],
                            ALU.subtract, ALU.mult)
    if gamma is not None:
        nc.vector.tensor_mul(src[:], src[:], gamma[:])
    if beta is not None:
        nc.vector.tensor_add(src[:], src[:], beta[:])


def build_nc(identity=False):
    """identity=True folds away LN gamma/beta and the bo2/bf2 bias adds,
    valid when g1=g2=1, b1=b2=bo2=bf2=0 (checked on the host)."""
    nc = bacc.Bacc(None, target_bir_lowering=False, debug=False)

    x_d = nc.dram_tensor("x", [S, E], F32, kind="ExternalInput")
    # host-preshuffled: row p holds M[o*128+p, :] for o in 0..7, concatenated
    m_d = nc.dram_tensor("Ms", [P, EB * E], F8, kind="ExternalInput")
    np_d = nc.dram_tensor("NPs", [P, EB * E], F8, kind="ExternalInput")
    # W1s[c, p, t*E + ei*128 + j] = W1[ei*128+p, (4c+t)*128+j]
    w1_d = nc.dram_tensor("W1s", [HB // 4, P, 4 * E], F8, kind="ExternalInput")
    # W2s[p, hb*E + n] = W2[hb*128+p, n]
    w2_d = nc.dram_tensor("W2s", [P, HB * E], F8, kind="ExternalInput")
    bo2_d = nc.dram_tensor("bo2", [E], F32, kind="ExternalInput")
    wrow_d = nc.dram_tensor("wrow", [S], F32, kind="ExternalInput")
    bf1_d = nc.dram_tensor("bf1", [F], F32, kind="ExternalInput")
    bf2_d = nc.dram_tensor("bf2", [E], F32, kind="ExternalInput")
    g1_d = nc.dram_tensor("g1", [E], F32, kind="ExternalInput")
    b1_d = nc.dram_tensor("b1", [E], F32, kind="ExternalInput")
    g2_d = nc.dram_tensor("g2", [E], F32, kind="ExternalInput")
    b2_d = nc.dram_tensor("b2", [E], F32, kind="ExternalInput")
    out_d = nc.dram_tensor("out", [S, E], F32, kind="ExternalOutput")
    h_d = nc.dram_tensor("h_scratch", [S, E], F32)   # LN1 output spill

    with tile.TileContext(nc, pool_alloc_mode="queue") as tc:
        with tc.tile_pool(name="const", bufs=1) as const:
            ident = const.tile([P, P], BF16)
            make_identity(nc, ident)
            ones_c = const.tile([P, 1], F32)
            nc.vector.memset(ones_c[:], 1.0)
            eps_c = const.tile([P, 1], F32)
            nc.vector.memset(eps_c[:], LN_EPS)
            # keep these strided gathers off the sync queue so the first
            # x-tile DMA isn't stuck behind them
            bf1_sb = const.tile([P, HB], F32)
            nc.gpsimd.dma_start(bf1_sb[:], bf1_d[:].rearrange("(o p) -> p o", p=P))
            recip_sb = const.tile([P, SB], F32)
            w_sb = const.tile([P, SB], F32)
            nc.gpsimd.dma_start(w_sb[:], wrow_d[:].rearrange("(o p) -> p o", p=P))

            with tc.tile_pool(name="pres", bufs=1) as pres:
                # resident through phase E: transposed LN1 output (built in
                # phase C, consumed in phase E — never leaves SBUF) and the
                # FFN weights, prefetched on the gpsimd queue from t=0.
                hT_sb = pres.tile([P, EB, S], F8)   # S_H * h^T
                w1_sb = pres.tile([P, HB // 4, 4, EB, P], F8)

                with tc.tile_pool(name="pbig", bufs=1) as pbig:
                    xT = pbig.tile([P, EB, S], F8)  # S_X*x^T

                    # ---- Phase A: load x, scale, transpose to xT ----------
                    with tc.tile_pool(name="pa", bufs=3) as pa, \
                         tc.tile_pool(name="pa_ps", bufs=4, space="PSUM") as pa_ps:
                        for sb in range(SB):
                            xf = pa.tile([P, E], F32, tag="xf")
                            nc.sync.dma_start(xf[:], x_d[sb * P:(sb + 1) * P, :])
                            xb = pa.tile([P, E], BF16, tag="xb")
                            nc.vector.tensor_scalar_mul(xb[:], xf[:], S_X)
                            for eb in range(EB):
                                pt = pa_ps.tile([P, P], BF16, tag="tp")
                                nc.tensor.transpose(
                                    pt[:], xb[:, eb * P:(eb + 1) * P], ident[:])
                                nc.scalar.copy(
                                    xT[:, eb, sb * P:(sb + 1) * P], pt[:])

                    with tc.tile_pool(name="pkv", bufs=1) as pkv:
                        AT = pkv.tile([P, EB, S], F8)   # S_A*(x@M)^T
                        VW = pkv.tile([P, SB, E], F8)   # S_VW*(x@NP), [k, f]

                        # ---- Phase B: AT, VW ------------------------------
                        with tc.tile_pool(name="wm", bufs=1) as wm, \
                             tc.tile_pool(name="pb_ps", bufs=4,
                                          space="PSUM") as pb_ps:
                            m_sb = wm.tile([P, EB, E], F8)
                            np_sb = wm.tile([P, EB, E], F8)
                            nc.scalar.dma_start(m_sb[:], m_d[:].rearrange(
                                "p (o n) -> p o n", n=E))
                            nc.scalar.dma_start(np_sb[:], np_d[:].rearrange(
                                "p (o n) -> p o n", n=E))
                            for eb in range(EB):
                                for qc in range(QC):
                                    ps = pb_ps.tile([P, NCHUNK], F32, tag="mm")
                                    for e2 in range(EB // 2):
                                        nc.tensor.matmul(
                                            ps[:],
                                            m_sb[:, 2 * e2:2 * e2 + 2,
                                                 eb * P:(eb + 1) * P],
                                            xT[:, 2 * e2:2 * e2 + 2,
                                               qc * NCHUNK:(qc + 1) * NCHUNK],
                                            start=(e2 == 0),
                                            stop=(e2 == EB // 2 - 1),
                                            perf_mode=DR)
                                    nc.scalar.activation(
                                        AT[:, eb, qc * NCHUNK:(qc + 1) * NCHUNK],
                                        ps[:], AF.Copy, scale=C_A)
                            for sb in range(SB):
                                for ec in range(E // NCHUNK):
                                    ps = pb_ps.tile([P, NCHUNK], F32, tag="mm")
                                    for e2 in range(EB // 2):
                                        nc.tensor.matmul(
                                            ps[:],
                                            xT[:, 2 * e2:2 * e2 + 2,
                                               sb * P:(sb + 1) * P],
                                            np_sb[:, 2 * e2:2 * e2 + 2,
                                                  ec * NCHUNK:(ec + 1) * NCHUNK],
                                            start=(e2 == 0),
                                            stop=(e2 == EB // 2 - 1),
                                            perf_mode=DR)
                                    nc.scalar.activation(
                                        VW[:, sb, ec * NCHUNK:(ec + 1) * NCHUNK],
                                        ps[:], AF.Copy, scale=C_VW)

                        # ---- Phase C: attention + proj, LN1 interleaved ---
                        with tc.tile_pool(name="pexp", bufs=2) as pexp, \
                             tc.tile_pool(name="pcw", bufs=1) as pcw, \
                             tc.tile_pool(name="pproj", bufs=2 * QPC) as pproj, \
                             tc.tile_pool(name="lnc", bufs=1) as lnc, \
                             tc.tile_pool(name="pdw", bufs=2) as pdw, \
                             tc.tile_pool(name="pc_ps", bufs=3,
                                          space="PSUM") as pc_ps, \
                             tc.tile_pool(name="pp_ps", bufs=2,
                                          space="PSUM") as pp_ps, \
                             tc.tile_pool(name="pr_ps", bufs=1,
                                          space="PSUM") as pr_ps, \
                             tc.tile_pool(name="pdt_ps", bufs=2,
                                          space="PSUM") as pdt_ps:
                            # W1 prefetch streams during phase C (issuing
                            # it earlier would steal DMA bandwidth from the
                            # phase-A x loads)
                            for c in range(HB // 4):
                                nc.gpsimd.dma_start(
                                    w1_sb[:, c], w1_d[c].rearrange(
                                        "p (t o n) -> p t o n", t=4, o=EB))
                            if identity:
                                bo2_b = g1_b = b1_b = None
                            else:
                                bo2_b = lnc.tile([P, E], F32)
                                g1_b = lnc.tile([P, E], F32)
                                b1_b = lnc.tile([P, E], F32)
                                nc.sync.dma_start(bo2_b[:], _bcast_ap(bo2_d[:]))
                                nc.sync.dma_start(g1_b[:], _bcast_ap(g1_d[:]))
                                nc.sync.dma_start(b1_b[:], _bcast_ap(b1_d[:]))
                            proj_tiles = {}

                            def d_chain(sb):
                                """residual + LN1 + transpose, one seq block."""
                                xf = pdw.tile([P, E], F32, tag="xres")
                                nc.sync.dma_start(xf[:],
                                                  x_d[sb * P:(sb + 1) * P, :])
                                hpre = pdw.tile([P, E], F32, tag="hpre")
                                nc.vector.tensor_scalar(hpre[:],
                                                        proj_tiles.pop(sb)[:],
                                                        recip_sb[:, sb:sb + 1],
                                                        None, ALU.mult)
                                if not identity:
                                    nc.vector.tensor_add(hpre[:], hpre[:],
                                                         bo2_b[:])
                                nc.vector.tensor_add(hpre[:], hpre[:], xf[:])
                                _layer_norm_inplace(nc, pdw, hpre, g1_b, b1_b,
                                                    eps_c)
                                nc.sync.dma_start(h_d[sb * P:(sb + 1) * P, :],
                                                  hpre[:])
                                hb16 = pdw.tile([P, E], BF16, tag="hb16")
                                nc.vector.tensor_scalar_mul(hb16[:], hpre[:], S_H)
                                for eb in range(EB):
                                    pt = pdt_ps.tile([P, P], BF16, tag="tp2")
                                    nc.tensor.transpose(
                                        pt[:], hb16[:, eb * P:(eb + 1) * P],
                                        ident[:])
                                    nc.scalar.copy(
                                        hT_sb[:, eb, sb * P:(sb + 1) * P], pt[:])

                            for qc in range(QC):
                                expS = pexp.tile([P, SB, NCHUNK], F8, tag="expS")
                                for kb in range(SB):
                                    ps = pc_ps.tile([P, NCHUNK], F32, tag="s")
                                    for e2 in range(EB // 2):
                                        nc.tensor.matmul(
                                            ps[:],
                                            xT[:, 2 * e2:2 * e2 + 2,
                                               kb * P:(kb + 1) * P],
                                            AT[:, 2 * e2:2 * e2 + 2,
                                               qc * NCHUNK:(qc + 1) * NCHUNK],
                                            start=(e2 == 0),
                                            stop=(e2 == EB // 2 - 1),
                                            perf_mode=DR)
                                    nc.scalar.activation(
                                        expS[:, kb, :], ps[:], AF.Exp,
                                        bias=w_sb[:, kb:kb + 1], scale=C_EXP)
                                if qc > 0:
                                    for qs in range(QPC):
                                        d_chain((qc - 1) * QPC + qs)
                                acc = [None] * 8
                                for j in range(8):
                                    a = pcw.tile([P, NCHUNK], F32, tag=f"acc{j}")
                                    nc.vector.tensor_add(a[:], expS[:, j, :],
                                                         expS[:, j + 8, :])
                                    acc[j] = a
                                for j in range(4):
                                    nc.vector.tensor_add(acc[j][:], acc[j][:],
                                                         acc[j + 4][:])
                                for j in range(2):
                                    nc.vector.tensor_add(acc[j][:], acc[j][:],
                                                         acc[j + 2][:])
                                nc.vector.tensor_add(acc[0][:], acc[0][:],
                                                     acc[1][:])
                                for qs in range(QPC):
                                    sb = qc * QPC + qs
                                    proj = pproj.tile([P, E], BF16, tag="proj")
                                    proj_tiles[sb] = proj
                                    for fc in range(E // NCHUNK):
                                        ps = pp_ps.tile([P, NCHUNK], F32,
                                                        tag="pp")
                                        for k2 in range(SB // 2):
                                            nc.tensor.matmul(
                                                ps[:],
                                                expS[:, 2 * k2:2 * k2 + 2,
                                                     qs * P:(qs + 1) * P],
                                                VW[:, 2 * k2:2 * k2 + 2,
                                                   fc * NCHUNK:(fc + 1) * NCHUNK],
                                                start=(k2 == 0),
                                                stop=(k2 == SB // 2 - 1),
                                                perf_mode=DR)
                                        nc.scalar.activation(
                                            proj[:, fc * NCHUNK:(fc + 1) * NCHUNK],
                                            ps[:], AF.Copy, scale=1.0 / S_VW)
                                for qs in range(QPC):
                                    sb = qc * QPC + qs
                                    pr = pr_ps.tile([P, 1], F32, tag="rs")
                                    nc.tensor.matmul(
                                        pr[:], acc[0][:, qs * P:(qs + 1) * P],
                                        ones_c[:], start=True, stop=True)
                                    nc.vector.reciprocal(recip_sb[:, sb:sb + 1],
                                                         pr[:])
                            for qs in range(QPC):
                                d_chain((QC - 1) * QPC + qs)
                    # pkv, pbig closed

                # ---- Phase E: FFN + LN2 + out -----------------------------
                with tc.tile_pool(name="w2r", bufs=1) as w2r, \
                     tc.tile_pool(name="lnc2", bufs=1) as lnc2, \
                     tc.tile_pool(name="pr1a", bufs=2) as pr1a, \
                     tc.tile_pool(name="pew", bufs=3) as pew, \
                     tc.tile_pool(name="pr1_ps", bufs=3, space="PSUM") as pr1_ps, \
                     tc.tile_pool(name="pf2_ps", bufs=4, space="PSUM") as pf2_ps:
                    w2_sb = w2r.tile([P, HB, E], F8)
                    w2_r = w2_d[:].rearrange("p (o n) -> p o n", n=E)
                    for hq in range(4):
                        nc.gpsimd.dma_start(
                            w2_sb[:, hq * (HB // 4):(hq + 1) * (HB // 4), :],
                            w2_r[:, hq * (HB // 4):(hq + 1) * (HB // 4), :])
                    if identity:
                        bf2_b = g2_b = b2_b = None
                    else:
                        bf2_b = lnc2.tile([P, E], F32)
                        g2_b = lnc2.tile([P, E], F32)
                        b2_b = lnc2.tile([P, E], F32)
                        nc.sync.dma_start(bf2_b[:], _bcast_ap(bf2_d[:]))
                        nc.sync.dma_start(g2_b[:], _bcast_ap(g2_d[:]))
                        nc.sync.dma_start(b2_b[:], _bcast_ap(b2_d[:]))
                    QW = 4 * P  # 4 seq blocks per group
                    for g in range(S // QW):
                        r1_all = pr1a.tile([P, HB, QW], F8, tag="r1a")
                        for c in range(HB // 4):
                            for t in range(4):
                                hb = c * 4 + t
                                ps1 = pr1_ps.tile([P, QW], F32, tag="r1")
                                for e2 in range(EB // 2):
                                    nc.tensor.matmul(
                                        ps1[:],
                                        w1_sb[:, c, t, 2 * e2:2 * e2 + 2, :],
                                        hT_sb[:, 2 * e2:2 * e2 + 2,
                                              g * QW:(g + 1) * QW],
                                        start=(e2 == 0),
                                        stop=(e2 == EB // 2 - 1),
                                        perf_mode=DR)
                                nc.scalar.activation(r1_all[:, hb, :], ps1[:],
                                                     AF.Relu,
                                                     bias=bf1_sb[:, hb:hb + 1],
                                                     scale=C_R1)
                        for i in range(QW // P):
                            sb = g * (QW // P) + i
                            t = pew.tile([P, E], F32, tag="ffn")
                            for j in range(E // NCHUNK):
                                ps = pf2_ps.tile([P, NCHUNK], F32, tag="f2")
                                for h2 in range(HB // 2):
                                    nc.tensor.matmul(
                                        ps[:],
                                        r1_all[:, 2 * h2:2 * h2 + 2,
                                               i * P:(i + 1) * P],
                                        w2_sb[:, 2 * h2:2 * h2 + 2,
                                              j * NCHUNK:(j + 1) * NCHUNK],
                                        start=(h2 == 0),
                                        stop=(h2 == HB // 2 - 1),
                                        perf_mode=DR)
                                nc.vector.tensor_scalar_mul(
                                    t[:, j * NCHUNK:(j + 1) * NCHUNK], ps[:],
                                    C_F2)
                            hres = pew.tile([P, E], F32, tag="hres")
                            nc.sync.dma_start(hres[:],
                                              h_d[sb * P:(sb + 1) * P, :])
                            if not identity:
                                nc.vector.tensor_add(hres[:], hres[:], bf2_b[:])
                            nc.vector.tensor_add(t[:], t[:], hres[:])
                            _layer_norm_inplace(nc, pew, t, g2_b, b2_b, eps_c)
                            nc.sync.dma_start(out_d[sb * P:(sb + 1) * P, :],
                                              t[:])

    nc.compile()
    return nc


def _get_nc(identity):
    if identity not in _CACHED_NC:
        _CACHED_NC[identity] = build_nc(identity)
    return _CACHED_NC[identity]


def kernel(**inputs):
    x = np.ascontiguousarray(np.asarray(inputs["x"], dtype=np.float32))
    B = x.shape[0]
    assert x.shape == (8, S, E), x.shape

    def q8(a, s):
        v = np.clip(np.asarray(a, np.float64) * s, -240.0, 240.0)
        return np.ascontiguousarray(v.astype(np.float32)
                                    .astype(ml_dtypes.float8_e4m3))

    def f32(a):
        return np.ascontiguousarray(np.asarray(a, dtype=np.float32))

    Wq = np.asarray(inputs["Wq"], np.float32)
    Wk = np.asarray(inputs["Wk"], np.float32)
    Wv = np.asarray(inputs["Wv"], np.float32)
    Wo = np.asarray(inputs["Wo"], np.float32)
    bq = np.asarray(inputs["bq"], np.float32)
    bk = np.asarray(inputs["bk"], np.float32)
    bv = np.asarray(inputs["bv"], np.float32)
    bo = np.asarray(inputs["bo"], np.float32)
    W1 = np.asarray(inputs["W1"], np.float32)
    W2 = np.asarray(inputs["W2"], np.float32)
    scale = np.float32(SCALE)

    M = Wq @ Wk.T
    NP_ = Wv @ Wo
    # shuffles: row p of Ms holds M[o*128+p, :] blocks concatenated over o
    Ms = q8(M.reshape(EB, P, E).transpose(1, 0, 2).reshape(P, EB * E), S_M)
    NPs = q8(NP_.reshape(EB, P, E).transpose(1, 0, 2).reshape(P, EB * E), S_NP)
    # W1s[c, p, t*E + ei*128 + j] = W1[ei*128+p, (4c+t)*128+j]
    W1s = q8(W1.reshape(EB, P, HB // 4, 4, P)
             .transpose(2, 1, 3, 0, 4).reshape(HB // 4, P, 4 * E), S_W1)
    W2s = q8(W2.reshape(HB, P, E).transpose(1, 0, 2).reshape(P, HB * E), S_W2)

    shared = {
        "Ms": Ms, "NPs": NPs, "W1s": W1s, "W2s": W2s,
        "bo2": f32(bo + bv @ Wo),
        "bf1": f32(np.asarray(inputs["bf1"], np.float32) * S_R),
        "bf2": f32(inputs["bf2"]),
        "g1": f32(inputs["g1"]), "b1": f32(inputs["b1"]),
        "g2": f32(inputs["g2"]), "b2": f32(inputs["b2"]),
    }
    vq = Wk @ bq
    cq = float(bq @ bk)
    lse = np.float32(np.log(S_E))
    in_maps = [
        {"x": x[c], "wrow": f32(scale * (x[c] @ vq) + scale * cq + lse),
         **shared}
        for c in range(B)
    ]

    identity = bool(
        np.all(np.asarray(inputs["g1"], np.float32) == 1.0)
        and np.all(np.asarray(inputs["b1"], np.float32) == 0.0)
        and np.all(np.asarray(inputs["g2"], np.float32) == 1.0)
        and np.all(np.asarray(inputs["b2"], np.float32) == 0.0)
        and np.all(shared["bo2"] == 0.0)
        and np.all(shared["bf2"] == 0.0)
    )
    nc = _get_nc(identity)
    trace = bool(int(os.environ.get("BERT_TRACE", "0")))
    res = run_bass_kernel_spmd(nc, in_maps, core_ids=list(range(B)), trace=trace)
    if trace and res.exec_time_ns is not None:
        print(f"HW exec time: {res.exec_time_ns} ns")
        kernel.last_exec_time_ns = res.exec_time_ns
        kernel.last_trace = res.instructions_and_trace
    return np.stack([res.results[c]["out"] for c in range(B)]).astype(np.float32)


# revision 30
# speedup vs baseline: 1.0469x; 1.0136x over previous
"""BERT encoder block on 8 Trainium2 NeuronCores.

Strategy: pure data parallelism — batch 8 is split one batch element per core
(no collectives). Each core runs the full encoder block on its [2048, 1024]
slice. All six big matmuls run in fp8 (TRN e4m3) DoubleRow mode (2 fp8
weights per PE cell -> 2x contraction per pass); accumulation and the
residual/LN stream are fp32.

Algebraic folds done on the host (softmax row-invariance absorbs the
query-side bias term):
  M   = Wq @ Wk^T          -> scores = x M x^T  (one fused tensor A = x@M)
  NP  = Wv @ Wo            -> attn_out @ Wo = softmax(S) @ (x@NP) + bo2
  bo2 = bo + bv @ Wo
  wrow = scale*(x @ (Wk@bq) + bq.bk) + ln(S_E)   (key-side score bias)

fp8 scaling: every fp8 tensor T is stored as s_T * T with a power-of-2
per-tensor scale chosen so absmax stays well under TRN e4m3's +-240 (and
above the 2^-6 subnormal threshold for typical values). All descales fold
into existing activation scale/bias parameters:
  xT   = S_X  * x^T            (scaled during the f32->bf16 pre-transpose copy)
  Ms   = S_M  * M, NPs = S_NP * NP, W1s = S_W1 * W1, W2s = S_W2 * W2  (host)
  AT   = S_A  * (x@M)^T        (PSUM copy scale = S_A/(S_M*S_X))
  VW   = S_VW * (x@NP)         (PSUM copy scale = S_VW/(S_X*S_NP))
  expS = S_E  * exp(..)        (ln S_E folded into wrow bias on host)
  proj = S_E * num             (PV psum copy scale 1/S_VW; the softmax
  normalization 1/(S_E*D) is applied per-row in d_chain on VectorE, which
  decouples the PV matmul stream from the denominator reduction)
  hT   = S_H  * h^T            (scaled during the f32->bf16 pre-transpose copy)
  r1   = S_R  * relu(..)       (relu scale = S_R/(S_W1*S_H), bias = S_R*bf1)
  FFN2 out descale = 1/(S_R*S_W2) via one tensor_scalar_mul.
The fp32 residual/LN stream is never scaled.

Attention runs in transposed score layout S^T[k,q]: softmax denominators are
ones-matmuls and proj = P @ (x@NP) lands directly in [q, f] layout. The
residual+LN1+h-transpose work is interleaved into the attention loop with a
one-chunk lag so TensorE never drains. The transposed LN1 output hT stays
resident in SBUF (no DRAM round-trip) and the FFN weights are prefetched on
the gpsimd DMA queue during phases A..C, so Phase E starts without stalls.
Weights are pre-shuffled on the host so every big DMA is one contiguous
segment per partition.

Self-contained: hardcodes shapes from the problem spec.
"""
import os

import numpy as np
import ml_dtypes

import concourse.bacc as bacc
import concourse.bass as bass
import concourse.tile as tile
import concourse.mybir as mybir
from concourse.bass_utils import run_bass_kernel_spmd
from concourse.masks import make_identity

P = 128
S = 2048          # sequence length per core
E = 1024          # embed
F = 4096          # ffn hidden
SB = S // P       # 16 seq blocks
EB = E // P       # 8 embed blocks
HB = F // P       # 32 ffn blocks
NCHUNK = 512
QC = S // NCHUNK  # 4 q chunks
QPC = NCHUNK // P  # 4 seq blocks per chunk
LN_EPS = 1e-5
SCALE = 1.0 / np.sqrt(np.float32(E))

# fp8 per-tensor scales (powers of 2; absmaxes measured on the fixed input
# distribution with ~2x safety margin under TRN e4m3's +-240)
S_X = 32.0    # x absmax ~5.4   -> 173
S_M = 2048.0  # M absmax ~.058  -> 118
S_NP = 2048.0  # NP absmax ~.053 -> 107
S_A = 32.0    # A absmax ~1.9   -> 62
S_VW = 64.0   # VW absmax ~1.8  -> 115
S_E = 16.0    # exp absmax ~7.5 -> 119
S_H = 32.0    # h absmax ~4.9   -> 158
S_W1 = 4096.0  # W1 absmax 1/32 -> 128
S_R = 16.0    # r1 absmax ~3.2  -> 50
S_W2 = 8192.0  # W2 absmax 1/64 -> 128

C_A = S_A / (S_M * S_X)        # AT copy scale
C_VW = S_VW / (S_X * S_NP)     # VW copy scale
C_EXP = float(SCALE) / (S_X * S_A)  # exp activation scale
C_R1 = S_R / (S_W1 * S_H)      # relu activation scale
C_F2 = 1.0 / (S_R * S_W2)      # FFN2 descale

F32 = mybir.dt.float32
BF16 = mybir.dt.bfloat16
F8 = mybir.dt.float8e4
AF = mybir.ActivationFunctionType
ALU = mybir.AluOpType
DR = mybir.MatmulPerfMode.DoubleRow

_CACHED_NC = {}


def _bcast_ap(ap, parts=P):
    """DRAM row-vector -> [parts, n] partition-broadcast access pattern."""
    return bass.AP(tensor=ap.tensor, offset=ap.offset,
                   ap=[[0, parts]] + [list(d) for d in ap.ap])


def _layer_norm_inplace(nc, work, src, gamma, beta, eps_c):
    """LN over free dim of src [P, E] fp32, in place.

    gamma/beta None means identity (fold for the common g=1, b=0 case).
    """
    stats = work.tile([P, 2, 6], F32, tag="ln_stats")
    nc.vector.bn_stats(stats[:, 0, :], src[:, 0:512])
    nc.vector.bn_stats(stats[:, 1, :], src[:, 512:1024])
    mv = work.tile([P, 2], F32, tag="ln_mv")
    nc.vector.bn_aggr(mv[:], stats[:])
    std = work.tile([P, 1], F32, tag="ln_std")
    nc.scalar.activation(std[:], mv[:, 1:2], AF.Sqrt, bias=eps_c[:], scale=1.0)
    rstd = work.tile([P, 1], F32, tag="ln_rstd")
    nc.vector.reciprocal(rstd[:], std[:])
    nc.vector.tensor_scalar(src[:], src[:], mv[:, 0:1], rstd[:],
                            ALU.subtract, ALU.mult)
    if gamma is not None:
        nc.vector.tensor_mul(src[:], src[:], gamma[:])
    if beta is not None:
        nc.vector.tensor_add(src[:], src[:], beta[:])


def build_nc(identity=False):
    """identity=True folds away LN gamma/beta and the bo2/bf2 bias adds,
    valid when g1=g2=1, b1=b2=bo2=bf2=0 (checked on the host)."""
    nc = bacc.Bacc(None, target_bir_lowering=False, debug=False)

    x_d = nc.dram_tensor("x", [S, E], F32, kind="ExternalInput")
    # host-preshuffled: row p holds M[o*128+p, :] for o in 0..7, concatenated
    m_d = nc.dram_tensor("Ms", [P, EB * E], F8, kind="ExternalInput")
    np_d = nc.dram_tensor("NPs", [P, EB * E], F8, kind="ExternalInput")
    # W1s[c, p, t*E + ei*128 + j] = W1[ei*128+p, (4c+t)*128+j]
    w1_d = nc.dram_tensor("W1s", [HB // 4, P, 4 * E], F8, kind="ExternalInput")
    # W2s[p, hb*E + n] = W2[hb*128+p, n]
    w2_d = nc.dram_tensor("W2s", [P, HB * E], F8, kind="ExternalInput")
    bo2_d = nc.dram_tensor("bo2", [E], F32, kind="ExternalInput")
    wrow_d = nc.dram_tensor("wrow", [S], F32, kind="ExternalInput")
    bf1_d = nc.dram_tensor("bf1", [F], F32, kind="ExternalInput")
    bf2_d = nc.dram_tensor("bf2", [E], F32, kind="ExternalInput")
    g1_d = nc.dram_tensor("g1", [E], F32, kind="ExternalInput")
    b1_d = nc.dram_tensor("b1", [E], F32, kind="ExternalInput")
    g2_d = nc.dram_tensor("g2", [E], F32, kind="ExternalInput")
    b2_d = nc.dram_tensor("b2", [E], F32, kind="ExternalInput")
    out_d = nc.dram_tensor("out", [S, E], F32, kind="ExternalOutput")
    h_d = nc.dram_tensor("h_scratch", [S, E], BF16)  # S_H * LN1 output spill

    with tile.TileContext(nc, pool_alloc_mode="queue") as tc:
        with tc.tile_pool(name="const", bufs=1) as const:
            ident = const.tile([P, P], BF16)
            make_identity(nc, ident)
            ones_c = const.tile([P, 1], F32)
            nc.vector.memset(ones_c[:], 1.0)
            eps_c = const.tile([P, 1], F32)
            nc.vector.memset(eps_c[:], LN_EPS)
            eps_h = const.tile([P, 1], F32)
            nc.vector.memset(eps_h[:], LN_EPS / (S_H * S_H))
            # keep these strided gathers off the sync queue so the first
            # x-tile DMA isn't stuck behind them
            bf1_sb = const.tile([P, HB], F32)
            nc.gpsimd.dma_start(bf1_sb[:], bf1_d[:].rearrange("(o p) -> p o", p=P))
            recip_sb = const.tile([P, SB], F32)
            w_sb = const.tile([P, SB], F32)
            nc.gpsimd.dma_start(w_sb[:], wrow_d[:].rearrange("(o p) -> p o", p=P))

            with tc.tile_pool(name="pres", bufs=1) as pres:
                # resident through phase E: transposed LN1 output (built in
                # phase C, consumed in phase E — never leaves SBUF) and the
                # FFN weights, prefetched on the gpsimd queue from t=0.
                hT_sb = pres.tile([P, EB, S], F8)   # S_H * h^T
                w1_sb = pres.tile([P, HB // 4, 4, EB, P], F8)

                with tc.tile_pool(name="pbig", bufs=1) as pbig:
                    xT = pbig.tile([P, EB, S], F8)  # S_X*x^T

                    # ---- Phase A: load x, scale, transpose to xT ----------
                    with tc.tile_pool(name="pa", bufs=3) as pa, \
                         tc.tile_pool(name="pa_ps", bufs=4, space="PSUM") as pa_ps:
                        for sb in range(SB):
                            xf = pa.tile([P, E], F32, tag="xf")
                            nc.sync.dma_start(xf[:], x_d[sb * P:(sb + 1) * P, :])
                            xb = pa.tile([P, E], BF16, tag="xb")
                            nc.vector.tensor_scalar_mul(xb[:], xf[:], S_X)
                            for eb in range(EB):
                                pt = pa_ps.tile([P, P], BF16, tag="tp")
                                nc.tensor.transpose(
                                    pt[:], xb[:, eb * P:(eb + 1) * P], ident[:])
                                nc.scalar.copy(
                                    xT[:, eb, sb * P:(sb + 1) * P], pt[:])

                    with tc.tile_pool(name="pkv", bufs=1) as pkv:
                        AT = pkv.tile([P, EB, S], F8)   # S_A*(x@M)^T
                        VW = pkv.tile([P, SB, E], F8)   # S_VW*(x@NP), [k, f]

                        # ---- Phase B: AT, VW ------------------------------
                        with tc.tile_pool(name="wm", bufs=1) as wm, \
                             tc.tile_pool(name="pb_ps", bufs=4,
                                          space="PSUM") as pb_ps:
                            m_sb = wm.tile([P, EB, E], F8)
                            np_sb = wm.tile([P, EB, E], F8)
                            nc.scalar.dma_start(m_sb[:], m_d[:].rearrange(
                                "p (o n) -> p o n", n=E))
                            nc.scalar.dma_start(np_sb[:], np_d[:].rearrange(
                                "p (o n) -> p o n", n=E))
                            for eb in range(EB):
                                for qc in range(QC):
                                    ps = pb_ps.tile([P, NCHUNK], F32, tag="mm")
                                    for e2 in range(EB // 2):
                                        nc.tensor.matmul(
                                            ps[:],
                                            m_sb[:, 2 * e2:2 * e2 + 2,
                                                 eb * P:(eb + 1) * P],
                                            xT[:, 2 * e2:2 * e2 + 2,
                                               qc * NCHUNK:(qc + 1) * NCHUNK],
                                            start=(e2 == 0),
                                            stop=(e2 == EB // 2 - 1),
                                            perf_mode=DR)
                                    nc.scalar.activation(
                                        AT[:, eb, qc * NCHUNK:(qc + 1) * NCHUNK],
                                        ps[:], AF.Copy, scale=C_A)
                            for sb in range(SB):
                                for ec in range(E // NCHUNK):
                                    ps = pb_ps.tile([P, NCHUNK], F32, tag="mm")
                                    for e2 in range(EB // 2):
                                        nc.tensor.matmul(
                                            ps[:],
                                            xT[:, 2 * e2:2 * e2 + 2,
                                               sb * P:(sb + 1) * P],
                                            np_sb[:, 2 * e2:2 * e2 + 2,
                                                  ec * NCHUNK:(ec + 1) * NCHUNK],
                                            start=(e2 == 0),
                                            stop=(e2 == EB // 2 - 1),
                                            perf_mode=DR)
                                    nc.scalar.activation(
                                        VW[:, sb, ec * NCHUNK:(ec + 1) * NCHUNK],
                                        ps[:], AF.Copy, scale=C_VW)

                        # ---- Phase C: attention + proj, LN1 interleaved ---
                        with tc.tile_pool(name="pexp", bufs=2) as pexp, \
                             tc.tile_pool(name="pcw", bufs=1) as pcw, \
                             tc.tile_pool(name="pproj", bufs=2 * QPC) as pproj, \
                             tc.tile_pool(name="lnc", bufs=1) as lnc, \
                             tc.tile_pool(name="pdw", bufs=2) as pdw, \
                             tc.tile_pool(name="pc_ps", bufs=3,
                                          space="PSUM") as pc_ps, \
                             tc.tile_pool(name="pp_ps", bufs=2,
                                          space="PSUM") as pp_ps, \
                             tc.tile_pool(name="pr_ps", bufs=1,
                                          space="PSUM") as pr_ps, \
                             tc.tile_pool(name="pdt_ps", bufs=2,
                                          space="PSUM") as pdt_ps:
                            # W1 prefetch streams during phase C (issuing
                            # it earlier would steal DMA bandwidth from the
                            # phase-A x loads)
                            for c in range(HB // 4):
                                nc.gpsimd.dma_start(
                                    w1_sb[:, c], w1_d[c].rearrange(
                                        "p (t o n) -> p t o n", t=4, o=EB))
                            if identity:
                                bo2_b = g1_b = b1_b = None
                            else:
                                bo2_b = lnc.tile([P, E], F32)
                                g1_b = lnc.tile([P, E], F32)
                                b1_b = lnc.tile([P, E], F32)
                                nc.sync.dma_start(bo2_b[:], _bcast_ap(bo2_d[:]))
                                nc.sync.dma_start(g1_b[:], _bcast_ap(g1_d[:]))
                                nc.sync.dma_start(b1_b[:], _bcast_ap(b1_d[:]))
                                nc.vector.tensor_scalar_mul(g1_b[:], g1_b[:],
                                                            S_H)
                                nc.vector.tensor_scalar_mul(b1_b[:], b1_b[:],
                                                            S_H)
                            proj_tiles = {}

                            def d_chain(sb):
                                """residual + LN1 + transpose, one seq block."""
                                xf = pdw.tile([P, E], F32, tag="xres")
                                nc.sync.dma_start(xf[:],
                                                  x_d[sb * P:(sb + 1) * P, :])
                                hpre = pdw.tile([P, E], F32, tag="hpre")
                                nc.vector.tensor_scalar(hpre[:],
                                                        proj_tiles.pop(sb)[:],
                                                        recip_sb[:, sb:sb + 1],
                                                        None, ALU.mult)
                                if not identity:
                                    nc.vector.tensor_add(hpre[:], hpre[:],
                                                         bo2_b[:])
                                nc.vector.tensor_add(hpre[:], hpre[:], xf[:])
                                # LN1 with S_H folded in: Sqrt activation
                                # scale gives std/S_H, so the final
                                # normalize emits bf16 S_H*h in one op.
                                stats = pdw.tile([P, 2, 6], F32, tag="ln_stats")
                                nc.vector.bn_stats(stats[:, 0, :],
                                                   hpre[:, 0:512])
                                nc.vector.bn_stats(stats[:, 1, :],
                                                   hpre[:, 512:1024])
                                mv = pdw.tile([P, 2], F32, tag="ln_mv")
                                nc.vector.bn_aggr(mv[:], stats[:])
                                std = pdw.tile([P, 1], F32, tag="ln_std")
                                nc.scalar.activation(std[:], mv[:, 1:2], AF.Sqrt,
                                                     bias=eps_h[:],
                                                     scale=1.0 / (S_H * S_H))
                                rstd = pdw.tile([P, 1], F32, tag="ln_rstd")
                                nc.vector.reciprocal(rstd[:], std[:])
                                hb16 = pdw.tile([P, E], BF16, tag="hb16")
                                if identity:
                                    nc.vector.tensor_scalar(hb16[:], hpre[:],
                                                            mv[:, 0:1], rstd[:],
                                                            ALU.subtract,
                                                            ALU.mult)
                                else:
                                    nc.vector.tensor_scalar(hpre[:], hpre[:],
                                                            mv[:, 0:1], rstd[:],
                                                            ALU.subtract,
                                                            ALU.mult)
                                    nc.vector.tensor_mul(hpre[:], hpre[:],
                                                         g1_b[:])
                                    nc.vector.tensor_add(hb16[:], hpre[:],
                                                         b1_b[:])
                                nc.sync.dma_start(h_d[sb * P:(sb + 1) * P, :],
                                                  hb16[:])
                                for eb in range(EB):
                                    pt = pdt_ps.tile([P, P], BF16, tag="tp2")
                                    nc.tensor.transpose(
                                        pt[:], hb16[:, eb * P:(eb + 1) * P],
                                        ident[:])
                                    nc.scalar.copy(
                                        hT_sb[:, eb, sb * P:(sb + 1) * P], pt[:])

                            for qc in range(QC):
                                expS = pexp.tile([P, SB, NCHUNK], F8, tag="expS")
                                for kb in range(SB):
                                    ps = pc_ps.tile([P, NCHUNK], F32, tag="s")
                                    for e2 in range(EB // 2):
                                        nc.tensor.matmul(
                                            ps[:],
                                            xT[:, 2 * e2:2 * e2 + 2,
                                               kb * P:(kb + 1) * P],
                                            AT[:, 2 * e2:2 * e2 + 2,
                                               qc * NCHUNK:(qc + 1) * NCHUNK],
                                            start=(e2 == 0),
                                            stop=(e2 == EB // 2 - 1),
                                            perf_mode=DR)
                                    nc.scalar.activation(
                                        expS[:, kb, :], ps[:], AF.Exp,
                                        bias=w_sb[:, kb:kb + 1], scale=C_EXP)
                                if qc > 0:
                                    for qs in range(QPC):
                                        d_chain((qc - 1) * QPC + qs)
                                acc = [None] * 8
                                for j in range(8):
                                    a = pcw.tile([P, NCHUNK], F32, tag=f"acc{j}")
                                    nc.vector.tensor_add(a[:], expS[:, j, :],
                                                         expS[:, j + 8, :])
                                    acc[j] = a
                                for j in range(4):
                                    nc.vector.tensor_add(acc[j][:], acc[j][:],
                                                         acc[j + 4][:])
                                for j in range(2):
                                    nc.vector.tensor_add(acc[j][:], acc[j][:],
                                                         acc[j + 2][:])
                                nc.vector.tensor_add(acc[0][:], acc[0][:],
                                                     acc[1][:])
                                for qs in range(QPC):
                                    sb = qc * QPC + qs
                                    proj = pproj.tile([P, E], BF16, tag="proj")
                                    proj_tiles[sb] = proj
                                    for fc in range(E // NCHUNK):
                                        ps = pp_ps.tile([P, NCHUNK], F32,
                                                        tag="pp")
                                        for k2 in range(SB // 2):
                                            nc.tensor.matmul(
                                                ps[:],
                                                expS[:, 2 * k2:2 * k2 + 2,
                                                     qs * P:(qs + 1) * P],
                                                VW[:, 2 * k2:2 * k2 + 2,
                                                   fc * NCHUNK:(fc + 1) * NCHUNK],
                                                start=(k2 == 0),
                                                stop=(k2 == SB // 2 - 1),
                                                perf_mode=DR)
                                        nc.scalar.activation(
                                            proj[:, fc * NCHUNK:(fc + 1) * NCHUNK],
                                            ps[:], AF.Copy, scale=1.0 / S_VW)
                                for qs in range(QPC):
                                    sb = qc * QPC + qs
                                    pr = pr_ps.tile([P, 1], F32, tag="rs")
                                    nc.tensor.matmul(
                                        pr[:], acc[0][:, qs * P:(qs + 1) * P],
                                        ones_c[:], start=True, stop=True)
                                    nc.vector.reciprocal(recip_sb[:, sb:sb + 1],
                                                         pr[:])
                            for qs in range(QPC):
                                d_chain((QC - 1) * QPC + qs)
                    # pkv, pbig closed

                # ---- Phase E: FFN + LN2 + out -----------------------------
                with tc.tile_pool(name="w2r", bufs=1) as w2r, \
                     tc.tile_pool(name="lnc2", bufs=1) as lnc2, \
                     tc.tile_pool(name="pr1a", bufs=2) as pr1a, \
                     tc.tile_pool(name="pew", bufs=3) as pew, \
                     tc.tile_pool(name="pr1_ps", bufs=3, space="PSUM") as pr1_ps, \
                     tc.tile_pool(name="pf2_ps", bufs=4, space="PSUM") as pf2_ps:
                    w2_sb = w2r.tile([P, HB, E], F8)
                    w2_r = w2_d[:].rearrange("p (o n) -> p o n", n=E)
                    for hq in range(4):
                        nc.gpsimd.dma_start(
                            w2_sb[:, hq * (HB // 4):(hq + 1) * (HB // 4), :],
                            w2_r[:, hq * (HB // 4):(hq + 1) * (HB // 4), :])
                    if identity:
                        bf2_b = g2_b = b2_b = None
                    else:
                        bf2_b = lnc2.tile([P, E], F32)
                        g2_b = lnc2.tile([P, E], F32)
                        b2_b = lnc2.tile([P, E], F32)
                        nc.sync.dma_start(bf2_b[:], _bcast_ap(bf2_d[:]))
                        nc.vector.tensor_scalar_mul(bf2_b[:], bf2_b[:], S_H)
                        nc.sync.dma_start(g2_b[:], _bcast_ap(g2_d[:]))
                        nc.sync.dma_start(b2_b[:], _bcast_ap(b2_d[:]))
                    QW = 4 * P  # 4 seq blocks per group
                    for g in range(S // QW):
                        r1_all = pr1a.tile([P, HB, QW], F8, tag="r1a")
                        for c in range(HB // 4):
                            for t in range(4):
                                hb = c * 4 + t
                                ps1 = pr1_ps.tile([P, QW], F32, tag="r1")
                                for e2 in range(EB // 2):
                                    nc.tensor.matmul(
                                        ps1[:],
                                        w1_sb[:, c, t, 2 * e2:2 * e2 + 2, :],
                                        hT_sb[:, 2 * e2:2 * e2 + 2,
                                              g * QW:(g + 1) * QW],
                                        start=(e2 == 0),
                                        stop=(e2 == EB // 2 - 1),
                                        perf_mode=DR)
                                nc.scalar.activation(r1_all[:, hb, :], ps1[:],
                                                     AF.Relu,
                                                     bias=bf1_sb[:, hb:hb + 1],
                                                     scale=C_R1)
                        for i in range(QW // P):
                            sb = g * (QW // P) + i
                            t = pew.tile([P, E], F32, tag="ffn")
                            for j in range(E // NCHUNK):
                                ps = pf2_ps.tile([P, NCHUNK], F32, tag="f2")
                                for h2 in range(HB // 2):
                                    nc.tensor.matmul(
                                        ps[:],
                                        r1_all[:, 2 * h2:2 * h2 + 2,
                                               i * P:(i + 1) * P],
                                        w2_sb[:, 2 * h2:2 * h2 + 2,
                                              j * NCHUNK:(j + 1) * NCHUNK],
                                        start=(h2 == 0),
                                        stop=(h2 == HB // 2 - 1),
                                        perf_mode=DR)
                                nc.vector.tensor_scalar_mul(
                                    t[:, j * NCHUNK:(j + 1) * NCHUNK], ps[:],
                                    C_F2 * S_H)
                            hres = pew.tile([P, E], BF16, tag="hres")
                            nc.sync.dma_start(hres[:],
                                              h_d[sb * P:(sb + 1) * P, :])
                            if not identity:
                                nc.vector.tensor_add(hres[:], hres[:], bf2_b[:])
                            nc.vector.tensor_add(t[:], t[:], hres[:])
                            _layer_norm_inplace(nc, pew, t, g2_b, b2_b, eps_c)
                            nc.sync.dma_start(out_d[sb * P:(sb + 1) * P, :],
                                              t[:])

    nc.compile()
    return nc


def _get_nc(identity):
    if identity not in _CACHED_NC:
        _CACHED_NC[identity] = build_nc(identity)
    return _CACHED_NC[identity]


def kernel(**inputs):
    x = np.ascontiguousarray(np.asarray(inputs["x"], dtype=np.float32))
    B = x.shape[0]
    assert x.shape == (8, S, E), x.shape

    def q8(a, s):
        v = np.clip(np.asarray(a, np.float64) * s, -240.0, 240.0)
        return np.ascontiguousarray(v.astype(np.float32)
                                    .astype(ml_dtypes.float8_e4m3))

    def f32(a):
        return np.ascontiguousarray(np.asarray(a, dtype=np.float32))

    Wq = np.asarray(inputs["Wq"], np.float32)
    Wk = np.asarray(inputs["Wk"], np.float32)
    Wv = np.asarray(inputs["Wv"], np.float32)
    Wo = np.asarray(inputs["Wo"], np.float32)
    bq = np.asarray(inputs["bq"], np.float32)
    bk = np.asarray(inputs["bk"], np.float32)
    bv = np.asarray(inputs["bv"], np.float32)
    bo = np.asarray(inputs["bo"], np.float32)
    W1 = np.asarray(inputs["W1"], np.float32)
    W2 = np.asarray(inputs["W2"], np.float32)
    scale = np.float32(SCALE)

    M = Wq @ Wk.T
    NP_ = Wv @ Wo
    # shuffles: row p of Ms holds M[o*128+p, :] blocks concatenated over o
    Ms = q8(M.reshape(EB, P, E).transpose(1, 0, 2).reshape(P, EB * E), S_M)
    NPs = q8(NP_.reshape(EB, P, E).transpose(1, 0, 2).reshape(P, EB * E), S_NP)
    # W1s[c, p, t*E + ei*128 + j] = W1[ei*128+p, (4c+t)*128+j]
    W1s = q8(W1.reshape(EB, P, HB // 4, 4, P)
             .transpose(2, 1, 3, 0, 4).reshape(HB // 4, P, 4 * E), S_W1)
    W2s = q8(W2.reshape(HB, P, E).transpose(1, 0, 2).reshape(P, HB * E), S_W2)

    shared = {
        "Ms": Ms, "NPs": NPs, "W1s": W1s, "W2s": W2s,
        "bo2": f32(bo + bv @ Wo),
        "bf1": f32(np.asarray(inputs["bf1"], np.float32) * S_R),
        "bf2": f32(inputs["bf2"]),
        "g1": f32(inputs["g1"]), "b1": f32(inputs["b1"]),
        "g2": f32(inputs["g2"]), "b2": f32(inputs["b2"]),
    }
    vq = Wk @ bq
    cq = float(bq @ bk)
    lse = np.float32(np.log(S_E))
    in_maps = [
        {"x": x[c], "wrow": f32(scale * (x[c] @ vq) + scale * cq + lse),
         **shared}
        for c in range(B)
    ]

    identity = bool(
        np.all(np.asarray(inputs["g1"], np.float32) == 1.0)
        and np.all(np.asarray(inputs["b1"], np.float32) == 0.0)
        and np.all(np.asarray(inputs["g2"], np.float32) == 1.0)
        and np.all(np.asarray(inputs["b2"], np.float32) == 0.0)
        and np.all(shared["bo2"] == 0.0)
        and np.all(shared["bf2"] == 0.0)
    )
    nc = _get_nc(identity)
    trace = bool(int(os.environ.get("BERT_TRACE", "0")))
    res = run_bass_kernel_spmd(nc, in_maps, core_ids=list(range(B)), trace=trace)
    if trace and res.exec_time_ns is not None:
        print(f"HW exec time: {res.exec_time_ns} ns")
        kernel.last_exec_time_ns = res.exec_time_ns
        kernel.last_trace = res.instructions_and_trace
    return np.stack([res.results[c]["out"] for c in range(B)]).astype(np.float32)


# revision 31
# speedup vs baseline: 1.0553x; 1.0080x over previous
"""BERT encoder block on 8 Trainium2 NeuronCores.

Strategy: pure data parallelism — batch 8 is split one batch element per core
(no collectives). Each core runs the full encoder block on its [2048, 1024]
slice. All six big matmuls run in fp8 (TRN e4m3) DoubleRow mode (2 fp8
weights per PE cell -> 2x contraction per pass); accumulation and the
residual/LN stream are fp32.

Algebraic folds done on the host (softmax row-invariance absorbs the
query-side bias term):
  M   = Wq @ Wk^T          -> scores = x M x^T  (one fused tensor A = x@M)
  NP  = Wv @ Wo            -> attn_out @ Wo = softmax(S) @ (x@NP) + bo2
  bo2 = bo + bv @ Wo
  wrow = scale*(x @ (Wk@bq) + bq.bk) + ln(S_E)   (key-side score bias)

fp8 scaling: every fp8 tensor T is stored as s_T * T with a power-of-2
per-tensor scale chosen so absmax stays well under TRN e4m3's +-240 (and
above the 2^-6 subnormal threshold for typical values). All descales fold
into existing activation scale/bias parameters:
  xT   = S_X  * x^T            (scaled during the f32->bf16 pre-transpose copy)
  Ms   = S_M  * M, NPs = S_NP * NP, W1s = S_W1 * W1, W2s = S_W2 * W2  (host)
  AT   = S_A  * (x@M)^T        (PSUM copy scale = S_A/(S_M*S_X))
  VW   = S_VW * (x@NP)         (PSUM copy scale = S_VW/(S_X*S_NP))
  expS = S_E  * exp(..)        (ln S_E folded into wrow bias on host)
  proj = S_E * num             (PV psum copy scale 1/S_VW; the softmax
  normalization 1/(S_E*D) is applied per-row in d_chain on VectorE, which
  decouples the PV matmul stream from the denominator reduction)
  hT   = S_H  * h^T            (scaled during the f32->bf16 pre-transpose copy)
  r1   = S_R  * relu(..)       (relu scale = S_R/(S_W1*S_H), bias = S_R*bf1)
  FFN2 out descale = 1/(S_R*S_W2) via one tensor_scalar_mul.
The fp32 residual/LN stream is never scaled.

Attention runs in transposed score layout S^T[k,q]: softmax denominators are
ones-matmuls and proj = P @ (x@NP) lands directly in [q, f] layout. The
residual+LN1+h-transpose work is interleaved into the attention loop with a
one-chunk lag so TensorE never drains. The transposed LN1 output hT stays
resident in SBUF (no DRAM round-trip) and the FFN weights are prefetched on
the gpsimd DMA queue during phases A..C, so Phase E starts without stalls.
Weights are pre-shuffled on the host so every big DMA is one contiguous
segment per partition.

Self-contained: hardcodes shapes from the problem spec.
"""
import os

import numpy as np
import ml_dtypes

import concourse.bacc as bacc
import concourse.bass as bass
import concourse.tile as tile
import concourse.mybir as mybir
from concourse.bass_utils import run_bass_kernel_spmd
from concourse.masks import make_identity

P = 128
S = 2048          # sequence length per core
E = 1024          # embed
F = 4096          # ffn hidden
SB = S // P       # 16 seq blocks
EB = E // P       # 8 embed blocks
HB = F // P       # 32 ffn blocks
NCHUNK = 512
QC = S // NCHUNK  # 4 q chunks
QPC = NCHUNK // P  # 4 seq blocks per chunk
LN_EPS = 1e-5
SCALE = 1.0 / np.sqrt(np.float32(E))

# fp8 per-tensor scales (powers of 2; absmaxes measured on the fixed input
# distribution with ~2x safety margin under TRN e4m3's +-240)
S_X = 32.0    # x absmax ~5.4   -> 173
S_M = 2048.0  # M absmax ~.058  -> 118
S_NP = 2048.0  # NP absmax ~.053 -> 107
S_A = 32.0    # A absmax ~1.9   -> 62
S_VW = 64.0   # VW absmax ~1.8  -> 115
S_E = 16.0    # exp absmax ~7.5 -> 119
S_H = 32.0    # h absmax ~4.9   -> 158
S_W1 = 4096.0  # W1 absmax 1/32 -> 128
S_R = 16.0    # r1 absmax ~3.2  -> 50
S_W2 = 8192.0  # W2 absmax 1/64 -> 128

C_A = S_A / (S_M * S_X)        # AT copy scale
C_VW = S_VW / (S_X * S_NP)     # VW copy scale
C_EXP = float(SCALE) / (S_X * S_A)  # exp activation scale
C_R1 = S_R / (S_W1 * S_H)      # relu activation scale
C_F2 = 1.0 / (S_R * S_W2)      # FFN2 descale

F32 = mybir.dt.float32
BF16 = mybir.dt.bfloat16
F8 = mybir.dt.float8e4
AF = mybir.ActivationFunctionType
ALU = mybir.AluOpType
DR = mybir.MatmulPerfMode.DoubleRow

_CACHED_NC = {}


def _bcast_ap(ap, parts=P):
    """DRAM row-vector -> [parts, n] partition-broadcast access pattern."""
    return bass.AP(tensor=ap.tensor, offset=ap.offset,
                   ap=[[0, parts]] + [list(d) for d in ap.ap])


def _layer_norm_inplace(nc, work, src, gamma, beta, eps_c):
    """LN over free dim of src [P, E] fp32, in place.

    gamma/beta None means identity (fold for the common g=1, b=0 case).
    """
    stats = work.tile([P, 2, 6], F32, tag="ln_stats")
    nc.vector.bn_stats(stats[:, 0, :], src[:, 0:512])
    nc.vector.bn_stats(stats[:, 1, :], src[:, 512:1024])
    mv = work.tile([P, 2], F32, tag="ln_mv")
    nc.vector.bn_aggr(mv[:], stats[:])
    std = work.tile([P, 1], F32, tag="ln_std")
    nc.scalar.activation(std[:], mv[:, 1:2], AF.Sqrt, bias=eps_c[:], scale=1.0)
    rstd = work.tile([P, 1], F32, tag="ln_rstd")
    nc.vector.reciprocal(rstd[:], std[:])
    nc.vector.tensor_scalar(src[:], src[:], mv[:, 0:1], rstd[:],
                            ALU.subtract, ALU.mult)
    if gamma is not None:
        nc.vector.tensor_mul(src[:], src[:], gamma[:])
    if beta is not None:
        nc.vector.tensor_add(src[:], src[:], beta[:])


def build_nc(identity=False):
    """identity=True folds away LN gamma/beta and the bo2/bf2 bias adds,
    valid when g1=g2=1, b1=b2=bo2=bf2=0 (checked on the host)."""
    nc = bacc.Bacc(None, target_bir_lowering=False, debug=False)

    x_d = nc.dram_tensor("x", [S, E], F32, kind="ExternalInput")
    # host-preshuffled: row p holds M[o*128+p, :] for o in 0..7, concatenated
    m_d = nc.dram_tensor("Ms", [P, EB * E], F8, kind="ExternalInput")
    np_d = nc.dram_tensor("NPs", [P, EB * E], F8, kind="ExternalInput")
    # W1s[c, p, t*E + ei*128 + j] = W1[ei*128+p, (4c+t)*128+j]
    w1_d = nc.dram_tensor("W1s", [HB // 4, P, 4 * E], F8, kind="ExternalInput")
    # W2s[p, hb*E + n] = W2[hb*128+p, n]
    w2_d = nc.dram_tensor("W2s", [P, HB * E], F8, kind="ExternalInput")
    bo2_d = nc.dram_tensor("bo2", [E], F32, kind="ExternalInput")
    wrow_d = nc.dram_tensor("wrow", [P, SB], F32, kind="ExternalInput")
    bf1_d = nc.dram_tensor("bf1", [P, HB], F32, kind="ExternalInput")
    bf2_d = nc.dram_tensor("bf2", [E], F32, kind="ExternalInput")
    g1_d = nc.dram_tensor("g1", [E], F32, kind="ExternalInput")
    b1_d = nc.dram_tensor("b1", [E], F32, kind="ExternalInput")
    g2_d = nc.dram_tensor("g2", [E], F32, kind="ExternalInput")
    b2_d = nc.dram_tensor("b2", [E], F32, kind="ExternalInput")
    out_d = nc.dram_tensor("out", [S, E], F32, kind="ExternalOutput")
    h_d = nc.dram_tensor("h_scratch", [S, E], BF16)  # S_H * LN1 output spill

    with tile.TileContext(nc, pool_alloc_mode="queue") as tc:
        with tc.tile_pool(name="const", bufs=1) as const:
            ident = const.tile([P, P], BF16)
            make_identity(nc, ident)
            ones_c = const.tile([P, 1], F32)
            nc.vector.memset(ones_c[:], 1.0)
            eps_c = const.tile([P, 1], F32)
            nc.vector.memset(eps_c[:], LN_EPS)
            eps_h = const.tile([P, 1], F32)
            nc.vector.memset(eps_h[:], LN_EPS / (S_H * S_H))
            # keep these strided gathers off the sync queue so the first
            # x-tile DMA isn't stuck behind them
            bf1_sb = const.tile([P, HB], F32)
            nc.gpsimd.dma_start(bf1_sb[:], bf1_d[:])
            recip_sb = const.tile([P, SB], F32)
            w_sb = const.tile([P, SB], F32)
            nc.gpsimd.dma_start(w_sb[:], wrow_d[:])

            with tc.tile_pool(name="pres", bufs=1) as pres:
                # resident through phase E: transposed LN1 output (built in
                # phase C, consumed in phase E — never leaves SBUF) and the
                # FFN weights, prefetched on the gpsimd queue from t=0.
                hT_sb = pres.tile([P, EB, S], F8)   # S_H * h^T
                w1_sb = pres.tile([P, HB // 4, 4, EB, P], F8)

                with tc.tile_pool(name="pbig", bufs=1) as pbig:
                    xT = pbig.tile([P, EB, S], F8)  # S_X*x^T

                    # ---- Phase A: load x, scale, transpose to xT ----------
                    with tc.tile_pool(name="pa", bufs=3) as pa, \
                         tc.tile_pool(name="pa_ps", bufs=4, space="PSUM") as pa_ps:
                        for sb in range(SB):
                            xf = pa.tile([P, E], F32, tag="xf")
                            nc.sync.dma_start(xf[:], x_d[sb * P:(sb + 1) * P, :])
                            xb = pa.tile([P, E], BF16, tag="xb")
                            nc.vector.tensor_scalar_mul(xb[:], xf[:], S_X)
                            for eb in range(EB):
                                pt = pa_ps.tile([P, P], BF16, tag="tp")
                                nc.tensor.transpose(
                                    pt[:], xb[:, eb * P:(eb + 1) * P], ident[:])
                                nc.scalar.copy(
                                    xT[:, eb, sb * P:(sb + 1) * P], pt[:])

                    with tc.tile_pool(name="pkv", bufs=1) as pkv:
                        AT = pkv.tile([P, EB, S], F8)   # S_A*(x@M)^T
                        VW = pkv.tile([P, SB, E], F8)   # S_VW*(x@NP), [k, f]

                        # ---- Phase B: AT, VW ------------------------------
                        with tc.tile_pool(name="wm", bufs=1) as wm, \
                             tc.tile_pool(name="pb_ps", bufs=4,
                                          space="PSUM") as pb_ps:
                            m_sb = wm.tile([P, EB, E], F8)
                            np_sb = wm.tile([P, EB, E], F8)
                            nc.gpsimd.dma_start(m_sb[:], m_d[:].rearrange(
                                "p (o n) -> p o n", n=E))
                            nc.gpsimd.dma_start(np_sb[:], np_d[:].rearrange(
                                "p (o n) -> p o n", n=E))
                            for eb in range(EB):
                                for qc in range(QC):
                                    ps = pb_ps.tile([P, NCHUNK], F32, tag="mm")
                                    for e2 in range(EB // 2):
                                        nc.tensor.matmul(
                                            ps[:],
                                            m_sb[:, 2 * e2:2 * e2 + 2,
                                                 eb * P:(eb + 1) * P],
                                            xT[:, 2 * e2:2 * e2 + 2,
                                               qc * NCHUNK:(qc + 1) * NCHUNK],
                                            start=(e2 == 0),
                                            stop=(e2 == EB // 2 - 1),
                                            perf_mode=DR)
                                    nc.scalar.activation(
                                        AT[:, eb, qc * NCHUNK:(qc + 1) * NCHUNK],
                                        ps[:], AF.Copy, scale=C_A)
                            for sb in range(SB):
                                for ec in range(E // NCHUNK):
                                    ps = pb_ps.tile([P, NCHUNK], F32, tag="mm")
                                    for e2 in range(EB // 2):
                                        nc.tensor.matmul(
                                            ps[:],
                                            xT[:, 2 * e2:2 * e2 + 2,
                                               sb * P:(sb + 1) * P],
                                            np_sb[:, 2 * e2:2 * e2 + 2,
                                                  ec * NCHUNK:(ec + 1) * NCHUNK],
                                            start=(e2 == 0),
                                            stop=(e2 == EB // 2 - 1),
                                            perf_mode=DR)
                                    nc.scalar.activation(
                                        VW[:, sb, ec * NCHUNK:(ec + 1) * NCHUNK],
                                        ps[:], AF.Copy, scale=C_VW)

                        # ---- Phase C: attention + proj, LN1 interleaved ---
                        with tc.tile_pool(name="pexp", bufs=2) as pexp, \
                             tc.tile_pool(name="pcw", bufs=1) as pcw, \
                             tc.tile_pool(name="pproj", bufs=2 * QPC) as pproj, \
                             tc.tile_pool(name="lnc", bufs=1) as lnc, \
                             tc.tile_pool(name="pdw", bufs=2) as pdw, \
                             tc.tile_pool(name="pc_ps", bufs=3,
                                          space="PSUM") as pc_ps, \
                             tc.tile_pool(name="pp_ps", bufs=2,
                                          space="PSUM") as pp_ps, \
                             tc.tile_pool(name="pr_ps", bufs=1,
                                          space="PSUM") as pr_ps, \
                             tc.tile_pool(name="pdt_ps", bufs=2,
                                          space="PSUM") as pdt_ps:
                            # W1 prefetch streams during phase C (issuing
                            # it earlier would steal DMA bandwidth from the
                            # phase-A x loads)
                            for c in range(HB // 4):
                                nc.gpsimd.dma_start(
                                    w1_sb[:, c], w1_d[c].rearrange(
                                        "p (t o n) -> p t o n", t=4, o=EB))
                            if identity:
                                bo2_b = g1_b = b1_b = None
                            else:
                                bo2_b = lnc.tile([P, E], F32)
                                g1_b = lnc.tile([P, E], F32)
                                b1_b = lnc.tile([P, E], F32)
                                nc.sync.dma_start(bo2_b[:], _bcast_ap(bo2_d[:]))
                                nc.sync.dma_start(g1_b[:], _bcast_ap(g1_d[:]))
                                nc.sync.dma_start(b1_b[:], _bcast_ap(b1_d[:]))
                                nc.vector.tensor_scalar_mul(g1_b[:], g1_b[:],
                                                            S_H)
                                nc.vector.tensor_scalar_mul(b1_b[:], b1_b[:],
                                                            S_H)
                            proj_tiles = {}

                            def d_chain(sb):
                                """residual + LN1 + transpose, one seq block."""
                                xf = pdw.tile([P, E], F32, tag="xres")
                                nc.sync.dma_start(xf[:],
                                                  x_d[sb * P:(sb + 1) * P, :])
                                hpre = pdw.tile([P, E], F32, tag="hpre")
                                nc.vector.tensor_scalar(hpre[:],
                                                        proj_tiles.pop(sb)[:],
                                                        recip_sb[:, sb:sb + 1],
                                                        None, ALU.mult)
                                if not identity:
                                    nc.vector.tensor_add(hpre[:], hpre[:],
                                                         bo2_b[:])
                                nc.vector.tensor_add(hpre[:], hpre[:], xf[:])
                                # LN1 with S_H folded in: Sqrt activation
                                # scale gives std/S_H, so the final
                                # normalize emits bf16 S_H*h in one op.
                                stats = pdw.tile([P, 2, 6], F32, tag="ln_stats")
                                nc.vector.bn_stats(stats[:, 0, :],
                                                   hpre[:, 0:512])
                                nc.vector.bn_stats(stats[:, 1, :],
                                                   hpre[:, 512:1024])
                                mv = pdw.tile([P, 2], F32, tag="ln_mv")
                                nc.vector.bn_aggr(mv[:], stats[:])
                                std = pdw.tile([P, 1], F32, tag="ln_std")
                                nc.scalar.activation(std[:], mv[:, 1:2], AF.Sqrt,
                                                     bias=eps_h[:],
                                                     scale=1.0 / (S_H * S_H))
                                rstd = pdw.tile([P, 1], F32, tag="ln_rstd")
                                nc.vector.reciprocal(rstd[:], std[:])
                                hb16 = pdw.tile([P, E], BF16, tag="hb16")
                                if identity:
                                    nc.vector.tensor_scalar(hb16[:], hpre[:],
                                                            mv[:, 0:1], rstd[:],
                                                            ALU.subtract,
                                                            ALU.mult)
                                else:
                                    nc.vector.tensor_scalar(hpre[:], hpre[:],
                                                            mv[:, 0:1], rstd[:],
                                                            ALU.subtract,
                                                            ALU.mult)
                                    nc.vector.tensor_mul(hpre[:], hpre[:],
                                                         g1_b[:])
                                    nc.vector.tensor_add(hb16[:], hpre[:],
                                                         b1_b[:])
                                nc.sync.dma_start(h_d[sb * P:(sb + 1) * P, :],
                                                  hb16[:])
                                for eb in range(EB):
                                    pt = pdt_ps.tile([P, P], BF16, tag="tp2")
                                    nc.tensor.transpose(
                                        pt[:], hb16[:, eb * P:(eb + 1) * P],
                                        ident[:])
                                    nc.scalar.copy(
                                        hT_sb[:, eb, sb * P:(sb + 1) * P], pt[:])

                            for qc in range(QC):
                                expS = pexp.tile([P, SB, NCHUNK], F8, tag="expS")
                                for kb in range(SB):
                                    ps = pc_ps.tile([P, NCHUNK], F32, tag="s")
                                    for e2 in range(EB // 2):
                                        nc.tensor.matmul(
                                            ps[:],
                                            xT[:, 2 * e2:2 * e2 + 2,
                                               kb * P:(kb + 1) * P],
                                            AT[:, 2 * e2:2 * e2 + 2,
                                               qc * NCHUNK:(qc + 1) * NCHUNK],
                                            start=(e2 == 0),
                                            stop=(e2 == EB // 2 - 1),
                                            perf_mode=DR)
                                    nc.scalar.activation(
                                        expS[:, kb, :], ps[:], AF.Exp,
                                        bias=w_sb[:, kb:kb + 1], scale=C_EXP)
                                if qc > 0:
                                    for qs in range(QPC):
                                        d_chain((qc - 1) * QPC + qs)
                                acc = [None] * 8
                                for j in range(8):
                                    a = pcw.tile([P, NCHUNK], F32, tag=f"acc{j}")
                                    nc.vector.tensor_add(a[:], expS[:, j, :],
                                                         expS[:, j + 8, :])
                                    acc[j] = a
                                for j in range(4):
                                    nc.vector.tensor_add(acc[j][:], acc[j][:],
                                                         acc[j + 4][:])
                                for j in range(2):
                                    nc.vector.tensor_add(acc[j][:], acc[j][:],
                                                         acc[j + 2][:])
                                nc.vector.tensor_add(acc[0][:], acc[0][:],
                                                     acc[1][:])
                                for qs in range(QPC):
                                    sb = qc * QPC + qs
                                    proj = pproj.tile([P, E], BF16, tag="proj")
                                    proj_tiles[sb] = proj
                                    for fc in range(E // NCHUNK):
                                        ps = pp_ps.tile([P, NCHUNK], F32,
                                                        tag="pp")
                                        for k2 in range(SB // 2):
                                            nc.tensor.matmul(
                                                ps[:],
                                                expS[:, 2 * k2:2 * k2 + 2,
                                                     qs * P:(qs + 1) * P],
                                                VW[:, 2 * k2:2 * k2 + 2,
                                                   fc * NCHUNK:(fc + 1) * NCHUNK],
                                                start=(k2 == 0),
                                                stop=(k2 == SB // 2 - 1),
                                                perf_mode=DR)
                                        nc.scalar.activation(
                                            proj[:, fc * NCHUNK:(fc + 1) * NCHUNK],
                                            ps[:], AF.Copy, scale=1.0 / S_VW)
                                for qs in range(QPC):
                                    sb = qc * QPC + qs
                                    pr = pr_ps.tile([P, 1], F32, tag="rs")
                                    nc.tensor.matmul(
                                        pr[:], acc[0][:, qs * P:(qs + 1) * P],
                                        ones_c[:], start=True, stop=True)
                                    nc.vector.reciprocal(recip_sb[:, sb:sb + 1],
                                                         pr[:])
                            for qs in range(QPC):
                                d_chain((QC - 1) * QPC + qs)
                    # pkv, pbig closed

                # ---- Phase E: FFN + LN2 + out -----------------------------
                with tc.tile_pool(name="w2r", bufs=1) as w2r, \
                     tc.tile_pool(name="lnc2", bufs=1) as lnc2, \
                     tc.tile_pool(name="pr1a", bufs=2) as pr1a, \
                     tc.tile_pool(name="pew", bufs=3) as pew, \
                     tc.tile_pool(name="pr1_ps", bufs=3, space="PSUM") as pr1_ps, \
                     tc.tile_pool(name="pf2_ps", bufs=4, space="PSUM") as pf2_ps:
                    w2_sb = w2r.tile([P, HB, E], F8)
                    w2_r = w2_d[:].rearrange("p (o n) -> p o n", n=E)
                    for hq in range(4):
                        nc.gpsimd.dma_start(
                            w2_sb[:, hq * (HB // 4):(hq + 1) * (HB // 4), :],
                            w2_r[:, hq * (HB // 4):(hq + 1) * (HB // 4), :])
                    if identity:
                        bf2_b = g2_b = b2_b = None
                    else:
                        bf2_b = lnc2.tile([P, E], F32)
                        g2_b = lnc2.tile([P, E], F32)
                        b2_b = lnc2.tile([P, E], F32)
                        nc.sync.dma_start(bf2_b[:], _bcast_ap(bf2_d[:]))
                        nc.vector.tensor_scalar_mul(bf2_b[:], bf2_b[:], S_H)
                        nc.sync.dma_start(g2_b[:], _bcast_ap(g2_d[:]))
                        nc.sync.dma_start(b2_b[:], _bcast_ap(b2_d[:]))
                    QW = 4 * P  # 4 seq blocks per group
                    for g in range(S // QW):
                        r1_all = pr1a.tile([P, HB, QW], F8, tag="r1a")
                        for c in range(HB // 4):
                            for t in range(4):
                                hb = c * 4 + t
                                ps1 = pr1_ps.tile([P, QW], F32, tag="r1")
                                for e2 in range(EB // 2):
                                    nc.tensor.matmul(
                                        ps1[:],
                                        w1_sb[:, c, t, 2 * e2:2 * e2 + 2, :],
                                        hT_sb[:, 2 * e2:2 * e2 + 2,
                                              g * QW:(g + 1) * QW],
                                        start=(e2 == 0),
                                        stop=(e2 == EB // 2 - 1),
                                        perf_mode=DR)
                                nc.scalar.activation(r1_all[:, hb, :], ps1[:],
                                                     AF.Relu,
                                                     bias=bf1_sb[:, hb:hb + 1],
                                                     scale=C_R1)
                        for i in range(QW // P):
                            sb = g * (QW // P) + i
                            t = pew.tile([P, E], F32, tag="ffn")
                            for j in range(E // NCHUNK):
                                ps = pf2_ps.tile([P, NCHUNK], F32, tag="f2")
                                for h2 in range(HB // 2):
                                    nc.tensor.matmul(
                                        ps[:],
                                        r1_all[:, 2 * h2:2 * h2 + 2,
                                               i * P:(i + 1) * P],
                                        w2_sb[:, 2 * h2:2 * h2 + 2,
                                              j * NCHUNK:(j + 1) * NCHUNK],
                                        start=(h2 == 0),
                                        stop=(h2 == HB // 2 - 1),
                                        perf_mode=DR)
                                nc.vector.tensor_scalar_mul(
                                    t[:, j * NCHUNK:(j + 1) * NCHUNK], ps[:],
                                    C_F2 * S_H)
                            hres = pew.tile([P, E], BF16, tag="hres")
                            nc.sync.dma_start(hres[:],
                                              h_d[sb * P:(sb + 1) * P, :])
                            if not identity:
                                nc.vector.tensor_add(hres[:], hres[:], bf2_b[:])
                            stats = pew.tile([P, 2, 6], F32, tag="ln_stats")
                            for j in range(2):
                                sl = slice(j * 512, (j + 1) * 512)
                                nc.vector.tensor_add(t[:, sl], t[:, sl],
                                                     hres[:, sl])
                                nc.vector.bn_stats(stats[:, j, :], t[:, sl])
                            mv = pew.tile([P, 2], F32, tag="ln_mv")
                            nc.vector.bn_aggr(mv[:], stats[:])
                            std = pew.tile([P, 1], F32, tag="ln_std")
                            nc.scalar.activation(std[:], mv[:, 1:2], AF.Sqrt,
                                                 bias=eps_c[:], scale=1.0)
                            rstd = pew.tile([P, 1], F32, tag="ln_rstd")
                            nc.vector.reciprocal(rstd[:], std[:])
                            nc.vector.tensor_scalar(t[:], t[:], mv[:, 0:1],
                                                    rstd[:], ALU.subtract,
                                                    ALU.mult)
                            if not identity:
                                nc.vector.tensor_mul(t[:], t[:], g2_b[:])
                                nc.vector.tensor_add(t[:], t[:], b2_b[:])
                            nc.sync.dma_start(out_d[sb * P:(sb + 1) * P, :],
                                              t[:])

    nc.compile()
    return nc


def _get_nc(identity):
    if identity not in _CACHED_NC:
        _CACHED_NC[identity] = build_nc(identity)
    return _CACHED_NC[identity]


def kernel(**inputs):
    x = np.ascontiguousarray(np.asarray(inputs["x"], dtype=np.float32))
    B = x.shape[0]
    assert x.shape == (8, S, E), x.shape

    def q8(a, s):
        v = np.clip(np.asarray(a, np.float64) * s, -240.0, 240.0)
        return np.ascontiguousarray(v.astype(np.float32)
                                    .astype(ml_dtypes.float8_e4m3))

    def f32(a):
        return np.ascontiguousarray(np.asarray(a, dtype=np.float32))

    Wq = np.asarray(inputs["Wq"], np.float32)
    Wk = np.asarray(inputs["Wk"], np.float32)
    Wv = np.asarray(inputs["Wv"], np.float32)
    Wo = np.asarray(inputs["Wo"], np.float32)
    bq = np.asarray(inputs["bq"], np.float32)
    bk = np.asarray(inputs["bk"], np.float32)
    bv = np.asarray(inputs["bv"], np.float32)
    bo = np.asarray(inputs["bo"], np.float32)
    W1 = np.asarray(inputs["W1"], np.float32)
    W2 = np.asarray(inputs["W2"], np.float32)
    scale = np.float32(SCALE)

    M = Wq @ Wk.T
    NP_ = Wv @ Wo
    # shuffles: row p of Ms holds M[o*128+p, :] blocks concatenated over o
    Ms = q8(M.reshape(EB, P, E).transpose(1, 0, 2).reshape(P, EB * E), S_M)
    NPs = q8(NP_.reshape(EB, P, E).transpose(1, 0, 2).reshape(P, EB * E), S_NP)
    # W1s[c, p, t*E + ei*128 + j] = W1[ei*128+p, (4c+t)*128+j]
    W1s = q8(W1.reshape(EB, P, HB // 4, 4, P)
             .transpose(2, 1, 3, 0, 4).reshape(HB // 4, P, 4 * E), S_W1)
    W2s = q8(W2.reshape(HB, P, E).transpose(1, 0, 2).reshape(P, HB * E), S_W2)

    shared = {
        "Ms": Ms, "NPs": NPs, "W1s": W1s, "W2s": W2s,
        "bo2": f32(bo + bv @ Wo),
        "bf1": f32((np.asarray(inputs["bf1"], np.float32) * S_R)
                   .reshape(HB, P).T),
        "bf2": f32(inputs["bf2"]),
        "g1": f32(inputs["g1"]), "b1": f32(inputs["b1"]),
        "g2": f32(inputs["g2"]), "b2": f32(inputs["b2"]),
    }
    vq = Wk @ bq
    cq = float(bq @ bk)
    lse = np.float32(np.log(S_E))
    in_maps = [
        {"x": x[c],
         "wrow": f32((scale * (x[c] @ vq) + scale * cq + lse)
                     .reshape(SB, P).T),
         **shared}
        for c in range(B)
    ]

    identity = bool(
        np.all(np.asarray(inputs["g1"], np.float32) == 1.0)
        and np.all(np.asarray(inputs["b1"], np.float32) == 0.0)
        and np.all(np.asarray(inputs["g2"], np.float32) == 1.0)
        and np.all(np.asarray(inputs["b2"], np.float32) == 0.0)
        and np.all(shared["bo2"] == 0.0)
        and np.all(shared["bf2"] == 0.0)
    )
    nc = _get_nc(identity)
    trace = bool(int(os.environ.get("BERT_TRACE", "0")))
    res = run_bass_kernel_spmd(nc, in_maps, core_ids=list(range(B)), trace=trace)
    if trace and res.exec_time_ns is not None:
        print(f"HW exec time: {res.exec_time_ns} ns")
        kernel.last_exec_time_ns = res.exec_time_ns
        kernel.last_trace = res.instructions_and_trace
    return np.stack([res.results[c]["out"] for c in range(B)]).astype(np.float32)


# revision 33
# speedup vs baseline: 1.0586x; 1.0032x over previous
"""BERT encoder block on 8 Trainium2 NeuronCores.

Strategy: pure data parallelism — batch 8 is split one batch element per core
(no collectives). Each core runs the full encoder block on its [2048, 1024]
slice. All six big matmuls run in fp8 (TRN e4m3) DoubleRow mode (2 fp8
weights per PE cell -> 2x contraction per pass); accumulation and the
residual/LN stream are fp32.

Algebraic folds done on the host (softmax row-invariance absorbs the
query-side bias term):
  M   = Wq @ Wk^T          -> scores = x M x^T  (one fused tensor A = x@M)
  NP  = Wv @ Wo            -> attn_out @ Wo = softmax(S) @ (x@NP) + bo2
  bo2 = bo + bv @ Wo
  wrow = scale*(x @ (Wk@bq) + bq.bk) + ln(S_E)   (key-side score bias)

fp8 scaling: every fp8 tensor T is stored as s_T * T with a power-of-2
per-tensor scale chosen so absmax stays well under TRN e4m3's +-240 (and
above the 2^-6 subnormal threshold for typical values). All descales fold
into existing activation scale/bias parameters:
  xT   = S_X  * x^T            (scaled during the f32->bf16 pre-transpose copy)
  Ms   = S_M  * M, NPs = S_NP * NP, W1s = S_W1 * W1, W2s = S_W2 * W2  (host)
  AT   = S_A  * (x@M)^T        (PSUM copy scale = S_A/(S_M*S_X))
  VW   = S_VW * (x@NP)         (PSUM copy scale = S_VW/(S_X*S_NP))
  expS = S_E  * exp(..)        (ln S_E folded into wrow bias on host)
  proj = S_E * num             (PV psum copy scale 1/S_VW; the softmax
  normalization 1/(S_E*D) is applied per-row in d_chain on VectorE, which
  decouples the PV matmul stream from the denominator reduction)
  hT   = S_H  * h^T            (scaled during the f32->bf16 pre-transpose copy)
  r1   = S_R  * relu(..)       (relu scale = S_R/(S_W1*S_H), bias = S_R*bf1)
  FFN2 out descale = 1/(S_R*S_W2) via one tensor_scalar_mul.
The fp32 residual/LN stream is never scaled.

Attention runs in transposed score layout S^T[k,q]: softmax denominators are
ones-matmuls and proj = P @ (x@NP) lands directly in [q, f] layout. The
residual+LN1+h-transpose work is interleaved into the attention loop with a
one-chunk lag so TensorE never drains. The transposed LN1 output hT stays
resident in SBUF (no DRAM round-trip) and the FFN weights are prefetched on
the gpsimd DMA queue during phases A..C, so Phase E starts without stalls.
Weights are pre-shuffled on the host so every big DMA is one contiguous
segment per partition.

Self-contained: hardcodes shapes from the problem spec.
"""
import os

import numpy as np
import ml_dtypes

import concourse.bacc as bacc
import concourse.bass as bass
import concourse.tile as tile
import concourse.mybir as mybir
from concourse.bass_utils import run_bass_kernel_spmd
from concourse.masks import make_identity

P = 128
S = 2048          # sequence length per core
E = 1024          # embed
F = 4096          # ffn hidden
SB = S // P       # 16 seq blocks
EB = E // P       # 8 embed blocks
HB = F // P       # 32 ffn blocks
NCHUNK = 512
QC = S // NCHUNK  # 4 q chunks
QPC = NCHUNK // P  # 4 seq blocks per chunk
LN_EPS = 1e-5
SCALE = 1.0 / np.sqrt(np.float32(E))

# fp8 per-tensor scales (powers of 2; absmaxes measured on the fixed input
# distribution with ~2x safety margin under TRN e4m3's +-240)
S_X = 32.0    # x absmax ~5.4   -> 173
S_M = 2048.0  # M absmax ~.058  -> 118
S_NP = 2048.0  # NP absmax ~.053 -> 107
S_A = 32.0    # A absmax ~1.9   -> 62
S_VW = 64.0   # VW absmax ~1.8  -> 115
S_E = 16.0    # exp absmax ~7.5 -> 119
S_H = 32.0    # h absmax ~4.9   -> 158
S_W1 = 4096.0  # W1 absmax 1/32 -> 128
S_R = 16.0    # r1 absmax ~3.2  -> 50
S_W2 = 8192.0  # W2 absmax 1/64 -> 128

C_A = S_A / (S_M * S_X)        # AT copy scale
C_VW = S_VW / (S_X * S_NP)     # VW copy scale
C_EXP = float(SCALE) / (S_X * S_A)  # exp activation scale
C_R1 = S_R / (S_W1 * S_H)      # relu activation scale
C_F2 = 1.0 / (S_R * S_W2)      # FFN2 descale

F32 = mybir.dt.float32
BF16 = mybir.dt.bfloat16
F8 = mybir.dt.float8e4
AF = mybir.ActivationFunctionType
ALU = mybir.AluOpType
DR = mybir.MatmulPerfMode.DoubleRow

_CACHED_NC = {}


def _bcast_ap(ap, parts=P):
    """DRAM row-vector -> [parts, n] partition-broadcast access pattern."""
    return bass.AP(tensor=ap.tensor, offset=ap.offset,
                   ap=[[0, parts]] + [list(d) for d in ap.ap])


def _layer_norm_inplace(nc, work, src, gamma, beta, eps_c):
    """LN over free dim of src [P, E] fp32, in place.

    gamma/beta None means identity (fold for the common g=1, b=0 case).
    """
    stats = work.tile([P, 2, 6], F32, tag="ln_stats")
    nc.vector.bn_stats(stats[:, 0, :], src[:, 0:512])
    nc.vector.bn_stats(stats[:, 1, :], src[:, 512:1024])
    mv = work.tile([P, 2], F32, tag="ln_mv")
    nc.vector.bn_aggr(mv[:], stats[:])
    std = work.tile([P, 1], F32, tag="ln_std")
    nc.scalar.activation(std[:], mv[:, 1:2], AF.Sqrt, bias=eps_c[:], scale=1.0)
    rstd = work.tile([P, 1], F32, tag="ln_rstd")
    nc.vector.reciprocal(rstd[:], std[:])
    nc.vector.tensor_scalar(src[:], src[:], mv[:, 0:1], rstd[:],
                            ALU.subtract, ALU.mult)
    if gamma is not None:
        nc.vector.tensor_mul(src[:], src[:], gamma[:])
    if beta is not None:
        nc.vector.tensor_add(src[:], src[:], beta[:])


def build_nc(identity=False):
    """identity=True folds away LN gamma/beta and the bo2/bf2 bias adds,
    valid when g1=g2=1, b1=b2=bo2=bf2=0 (checked on the host)."""
    nc = bacc.Bacc(None, target_bir_lowering=False, debug=False)

    x_d = nc.dram_tensor("x", [S, E], F32, kind="ExternalInput")
    # host-preshuffled: row p holds M[o*128+p, :] for o in 0..7, concatenated
    m_d = nc.dram_tensor("Ms", [P, EB * E], F8, kind="ExternalInput")
    np_d = nc.dram_tensor("NPs", [P, EB * E], F8, kind="ExternalInput")
    # W1s[c, p, t*E + ei*128 + j] = W1[ei*128+p, (4c+t)*128+j]
    w1_d = nc.dram_tensor("W1s", [HB // 4, P, 4 * E], F8, kind="ExternalInput")
    # W2s[p, hb*E + n] = W2[hb*128+p, n]
    w2_d = nc.dram_tensor("W2s", [P, HB * E], F8, kind="ExternalInput")
    bo2_d = nc.dram_tensor("bo2", [E], F32, kind="ExternalInput")
    wrow_d = nc.dram_tensor("wrow", [P, SB], F32, kind="ExternalInput")
    bf1_d = nc.dram_tensor("bf1", [P, HB], F32, kind="ExternalInput")
    bf2_d = nc.dram_tensor("bf2", [E], F32, kind="ExternalInput")
    g1_d = nc.dram_tensor("g1", [E], F32, kind="ExternalInput")
    b1_d = nc.dram_tensor("b1", [E], F32, kind="ExternalInput")
    g2_d = nc.dram_tensor("g2", [E], F32, kind="ExternalInput")
    b2_d = nc.dram_tensor("b2", [E], F32, kind="ExternalInput")
    out_d = nc.dram_tensor("out", [S, E], F32, kind="ExternalOutput")
    h_d = nc.dram_tensor("h_scratch", [S, E], BF16)  # S_H * LN1 output spill

    with tile.TileContext(nc, pool_alloc_mode="queue") as tc:
        with tc.tile_pool(name="const", bufs=1) as const:
            ident = const.tile([P, P], BF16)
            make_identity(nc, ident)
            ones_c = const.tile([P, 1], F32)
            nc.vector.memset(ones_c[:], 1.0)
            eps_c = const.tile([P, 1], F32)
            nc.vector.memset(eps_c[:], LN_EPS)
            eps_h = const.tile([P, 1], F32)
            nc.vector.memset(eps_h[:], LN_EPS / (S_H * S_H))
            # keep these strided gathers off the sync queue so the first
            # x-tile DMA isn't stuck behind them
            bf1_sb = const.tile([P, HB], F32)
            nc.gpsimd.dma_start(bf1_sb[:], bf1_d[:])
            recip_sb = const.tile([P, SB], F32)
            w_sb = const.tile([P, SB], F32)
            nc.gpsimd.dma_start(w_sb[:], wrow_d[:])

            with tc.tile_pool(name="pres", bufs=1) as pres:
                # resident through phase E: transposed LN1 output (built in
                # phase C, consumed in phase E — never leaves SBUF) and the
                # FFN weights, prefetched on the gpsimd queue from t=0.
                hT_sb = pres.tile([P, EB, S], F8)   # S_H * h^T
                w1_sb = pres.tile([P, HB // 4, 4, EB, P], F8)

                with tc.tile_pool(name="pbig", bufs=1) as pbig:
                    xT = pbig.tile([P, EB, S], F8)  # S_X*x^T

                    # ---- Phase A: load x, scale, transpose to xT ----------
                    with tc.tile_pool(name="pa", bufs=3) as pa, \
                         tc.tile_pool(name="pa_ps", bufs=4, space="PSUM") as pa_ps:
                        for sb in range(SB):
                            xf = pa.tile([P, E], F32, tag="xf")
                            nc.sync.dma_start(xf[:], x_d[sb * P:(sb + 1) * P, :])
                            xb = pa.tile([P, E], BF16, tag="xb")
                            nc.vector.tensor_scalar_mul(xb[:], xf[:], S_X)
                            for eb in range(EB):
                                pt = pa_ps.tile([P, P], BF16, tag="tp")
                                nc.tensor.transpose(
                                    pt[:], xb[:, eb * P:(eb + 1) * P], ident[:])
                                nc.scalar.copy(
                                    xT[:, eb, sb * P:(sb + 1) * P], pt[:])

                    with tc.tile_pool(name="pkv", bufs=1) as pkv:
                        AT = pkv.tile([P, EB, S], F8)   # S_A*(x@M)^T
                        VW = pkv.tile([P, SB, E], F8)   # S_VW*(x@NP), [k, f]

                        # ---- Phase B: AT, VW ------------------------------
                        with tc.tile_pool(name="wm", bufs=1) as wm, \
                             tc.tile_pool(name="pb_ps", bufs=4,
                                          space="PSUM") as pb_ps:
                            m_sb = wm.tile([P, EB, E], F8)
                            np_sb = wm.tile([P, EB, E], F8)
                            nc.gpsimd.dma_start(m_sb[:], m_d[:].rearrange(
                                "p (o n) -> p o n", n=E))
                            nc.gpsimd.dma_start(np_sb[:], np_d[:].rearrange(
                                "p (o n) -> p o n", n=E))
                            for eb in range(EB):
                                for qc in range(QC):
                                    ps = pb_ps.tile([P, NCHUNK], F32, tag="mm")
                                    for e2 in range(EB // 2):
                                        nc.tensor.matmul(
                                            ps[:],
                                            m_sb[:, 2 * e2:2 * e2 + 2,
                                                 eb * P:(eb + 1) * P],
                                            xT[:, 2 * e2:2 * e2 + 2,
                                               qc * NCHUNK:(qc + 1) * NCHUNK],
                                            start=(e2 == 0),
                                            stop=(e2 == EB // 2 - 1),
                                            perf_mode=DR)
                                    nc.scalar.activation(
                                        AT[:, eb, qc * NCHUNK:(qc + 1) * NCHUNK],
                                        ps[:], AF.Copy, scale=C_A)
                            for sb in range(SB):
                                ps_a = pb_ps.tile([P, NCHUNK], F32, tag="mm")
                                ps_b = pb_ps.tile([P, NCHUNK], F32, tag="mm")
                                pss = [ps_a, ps_b]
                                for e2 in range(EB // 2):
                                    for ec in range(E // NCHUNK):
                                        nc.tensor.matmul(
                                            pss[ec][:],
                                            xT[:, 2 * e2:2 * e2 + 2,
                                               sb * P:(sb + 1) * P],
                                            np_sb[:, 2 * e2:2 * e2 + 2,
                                                  ec * NCHUNK:(ec + 1) * NCHUNK],
                                            start=(e2 == 0),
                                            stop=(e2 == EB // 2 - 1),
                                            perf_mode=DR)
                                for ec in range(E // NCHUNK):
                                    nc.scalar.activation(
                                        VW[:, sb, ec * NCHUNK:(ec + 1) * NCHUNK],
                                        pss[ec][:], AF.Copy, scale=C_VW)

                        # ---- Phase C: attention + proj, LN1 interleaved ---
                        with tc.tile_pool(name="pexp", bufs=2) as pexp, \
                             tc.tile_pool(name="pcw", bufs=1) as pcw, \
                             tc.tile_pool(name="pproj", bufs=2 * QPC) as pproj, \
                             tc.tile_pool(name="lnc", bufs=1) as lnc, \
                             tc.tile_pool(name="pdw", bufs=2) as pdw, \
                             tc.tile_pool(name="pc_ps", bufs=3,
                                          space="PSUM") as pc_ps, \
                             tc.tile_pool(name="pp_ps", bufs=2,
                                          space="PSUM") as pp_ps, \
                             tc.tile_pool(name="pr_ps", bufs=1,
                                          space="PSUM") as pr_ps, \
                             tc.tile_pool(name="pdt_ps", bufs=2,
                                          space="PSUM") as pdt_ps:
                            # W1 prefetch streams during phase C (issuing
                            # it earlier would steal DMA bandwidth from the
                            # phase-A x loads)
                            for c in range(HB // 4):
                                nc.gpsimd.dma_start(
                                    w1_sb[:, c], w1_d[c].rearrange(
                                        "p (t o n) -> p t o n", t=4, o=EB))
                            if identity:
                                bo2_b = g1_b = b1_b = None
                            else:
                                bo2_b = lnc.tile([P, E], F32)
                                g1_b = lnc.tile([P, E], F32)
                                b1_b = lnc.tile([P, E], F32)
                                nc.sync.dma_start(bo2_b[:], _bcast_ap(bo2_d[:]))
                                nc.sync.dma_start(g1_b[:], _bcast_ap(g1_d[:]))
                                nc.sync.dma_start(b1_b[:], _bcast_ap(b1_d[:]))
                                nc.vector.tensor_scalar_mul(g1_b[:], g1_b[:],
                                                            S_H)
                                nc.vector.tensor_scalar_mul(b1_b[:], b1_b[:],
                                                            S_H)
                            proj_tiles = {}

                            def d_chain(sb):
                                """residual + LN1 + transpose, one seq block."""
                                xf = pdw.tile([P, E], F32, tag="xres")
                                nc.sync.dma_start(xf[:],
                                                  x_d[sb * P:(sb + 1) * P, :])
                                hpre = pdw.tile([P, E], F32, tag="hpre")
                                nc.vector.tensor_scalar(hpre[:],
                                                        proj_tiles.pop(sb)[:],
                                                        recip_sb[:, sb:sb + 1],
                                                        None, ALU.mult)
                                if not identity:
                                    nc.vector.tensor_add(hpre[:], hpre[:],
                                                         bo2_b[:])
                                nc.vector.tensor_add(hpre[:], hpre[:], xf[:])
                                # LN1 with S_H folded in: Sqrt activation
                                # scale gives std/S_H, so the final
                                # normalize emits bf16 S_H*h in one op.
                                stats = pdw.tile([P, 2, 6], F32, tag="ln_stats")
                                nc.vector.bn_stats(stats[:, 0, :],
                                                   hpre[:, 0:512])
                                nc.vector.bn_stats(stats[:, 1, :],
                                                   hpre[:, 512:1024])
                                mv = pdw.tile([P, 2], F32, tag="ln_mv")
                                nc.vector.bn_aggr(mv[:], stats[:])
                                std = pdw.tile([P, 1], F32, tag="ln_std")
                                nc.scalar.activation(std[:], mv[:, 1:2], AF.Sqrt,
                                                     bias=eps_h[:],
                                                     scale=1.0 / (S_H * S_H))
                                rstd = pdw.tile([P, 1], F32, tag="ln_rstd")
                                nc.vector.reciprocal(rstd[:], std[:])
                                hb16 = pdw.tile([P, E], BF16, tag="hb16")
                                if identity:
                                    nc.vector.tensor_scalar(hb16[:], hpre[:],
                                                            mv[:, 0:1], rstd[:],
                                                            ALU.subtract,
                                                            ALU.mult)
                                else:
                                    nc.vector.tensor_scalar(hpre[:], hpre[:],
                                                            mv[:, 0:1], rstd[:],
                                                            ALU.subtract,
                                                            ALU.mult)
                                    nc.vector.tensor_mul(hpre[:], hpre[:],
                                                         g1_b[:])
                                    nc.vector.tensor_add(hb16[:], hpre[:],
                                                         b1_b[:])
                                nc.sync.dma_start(h_d[sb * P:(sb + 1) * P, :],
                                                  hb16[:])
                                for eb in range(EB):
                                    pt = pdt_ps.tile([P, P], BF16, tag="tp2")
                                    nc.tensor.transpose(
                                        pt[:], hb16[:, eb * P:(eb + 1) * P],
                                        ident[:])
                                    nc.scalar.copy(
                                        hT_sb[:, eb, sb * P:(sb + 1) * P], pt[:])

                            for qc in range(QC):
                                expS = pexp.tile([P, SB, NCHUNK], F8, tag="expS")
                                for kb in range(SB):
                                    ps = pc_ps.tile([P, NCHUNK], F32, tag="s")
                                    for e2 in range(EB // 2):
                                        nc.tensor.matmul(
                                            ps[:],
                                            xT[:, 2 * e2:2 * e2 + 2,
                                               kb * P:(kb + 1) * P],
                                            AT[:, 2 * e2:2 * e2 + 2,
                                               qc * NCHUNK:(qc + 1) * NCHUNK],
                                            start=(e2 == 0),
                                            stop=(e2 == EB // 2 - 1),
                                            perf_mode=DR)
                                    nc.scalar.activation(
                                        expS[:, kb, :], ps[:], AF.Exp,
                                        bias=w_sb[:, kb:kb + 1], scale=C_EXP)
                                if qc > 0:
                                    for qs in range(QPC):
                                        d_chain((qc - 1) * QPC + qs)
                                acc = [None] * 8
                                for j in range(8):
                                    a = pcw.tile([P, NCHUNK], F32, tag=f"acc{j}")
                                    nc.vector.tensor_add(a[:], expS[:, j, :],
                                                         expS[:, j + 8, :])
                                    acc[j] = a
                                for j in range(4):
                                    nc.vector.tensor_add(acc[j][:], acc[j][:],
                                                         acc[j + 4][:])
                                for j in range(2):
                                    nc.vector.tensor_add(acc[j][:], acc[j][:],
                                                         acc[j + 2][:])
                                nc.vector.tensor_add(acc[0][:], acc[0][:],
                                                     acc[1][:])
                                for qs in range(QPC):
                                    sb = qc * QPC + qs
                                    proj = pproj.tile([P, E], BF16, tag="proj")
                                    proj_tiles[sb] = proj
                                    for fc in range(E // NCHUNK):
                                        ps = pp_ps.tile([P, NCHUNK], F32,
                                                        tag="pp")
                                        for k2 in range(SB // 2):
                                            nc.tensor.matmul(
                                                ps[:],
                                                expS[:, 2 * k2:2 * k2 + 2,
                                                     qs * P:(qs + 1) * P],
                                                VW[:, 2 * k2:2 * k2 + 2,
                                                   fc * NCHUNK:(fc + 1) * NCHUNK],
                                                start=(k2 == 0),
                                                stop=(k2 == SB // 2 - 1),
                                                perf_mode=DR)
                                        nc.scalar.activation(
                                            proj[:, fc * NCHUNK:(fc + 1) * NCHUNK],
                                            ps[:], AF.Copy, scale=1.0 / S_VW)
                                for qs in range(QPC):
                                    sb = qc * QPC + qs
                                    pr = pr_ps.tile([P, 1], F32, tag="rs")
                                    nc.tensor.matmul(
                                        pr[:], acc[0][:, qs * P:(qs + 1) * P],
                                        ones_c[:], start=True, stop=True)
                                    nc.vector.reciprocal(recip_sb[:, sb:sb + 1],
                                                         pr[:])
                            for qs in range(QPC):
                                d_chain((QC - 1) * QPC + qs)
                    # pkv, pbig closed

                # ---- Phase E: FFN + LN2 + out -----------------------------
                with tc.tile_pool(name="w2r", bufs=1) as w2r, \
                     tc.tile_pool(name="lnc2", bufs=1) as lnc2, \
                     tc.tile_pool(name="pr1a", bufs=2) as pr1a, \
                     tc.tile_pool(name="pew", bufs=3) as pew, \
                     tc.tile_pool(name="pr1_ps", bufs=3, space="PSUM") as pr1_ps, \
                     tc.tile_pool(name="pf2_ps", bufs=4, space="PSUM") as pf2_ps:
                    w2_sb = w2r.tile([P, HB, E], F8)
                    w2_r = w2_d[:].rearrange("p (o n) -> p o n", n=E)
                    for hq in range(4):
                        nc.gpsimd.dma_start(
                            w2_sb[:, hq * (HB // 4):(hq + 1) * (HB // 4), :],
                            w2_r[:, hq * (HB // 4):(hq + 1) * (HB // 4), :])
                    if identity:
                        bf2_b = g2_b = b2_b = None
                    else:
                        bf2_b = lnc2.tile([P, E], F32)
                        g2_b = lnc2.tile([P, E], F32)
                        b2_b = lnc2.tile([P, E], F32)
                        nc.sync.dma_start(bf2_b[:], _bcast_ap(bf2_d[:]))
                        nc.vector.tensor_scalar_mul(bf2_b[:], bf2_b[:], S_H)
                        nc.sync.dma_start(g2_b[:], _bcast_ap(g2_d[:]))
                        nc.sync.dma_start(b2_b[:], _bcast_ap(b2_d[:]))
                    QW = 4 * P  # 4 seq blocks per group
                    for g in range(S // QW):
                        r1_all = pr1a.tile([P, HB, QW], F8, tag="r1a")
                        for c in range(HB // 4):
                            for t in range(4):
                                hb = c * 4 + t
                                ps1 = pr1_ps.tile([P, QW], F32, tag="r1")
                                for e2 in range(EB // 2):
                                    nc.tensor.matmul(
                                        ps1[:],
                                        w1_sb[:, c, t, 2 * e2:2 * e2 + 2, :],
                                        hT_sb[:, 2 * e2:2 * e2 + 2,
                                              g * QW:(g + 1) * QW],
                                        start=(e2 == 0),
                                        stop=(e2 == EB // 2 - 1),
                                        perf_mode=DR)
                                nc.scalar.activation(r1_all[:, hb, :], ps1[:],
                                                     AF.Relu,
                                                     bias=bf1_sb[:, hb:hb + 1],
                                                     scale=C_R1)
                        for i in range(QW // P):
                            sb = g * (QW // P) + i
                            t = pew.tile([P, E], F32, tag="ffn")
                            ps_a = pf2_ps.tile([P, NCHUNK], F32, tag="f2")
                            ps_b = pf2_ps.tile([P, NCHUNK], F32, tag="f2")
                            pss = [ps_a, ps_b]
                            for h2 in range(HB // 2):
                                for j in range(E // NCHUNK):
                                    nc.tensor.matmul(
                                        pss[j][:],
                                        r1_all[:, 2 * h2:2 * h2 + 2,
                                               i * P:(i + 1) * P],
                                        w2_sb[:, 2 * h2:2 * h2 + 2,
                                              j * NCHUNK:(j + 1) * NCHUNK],
                                        start=(h2 == 0),
                                        stop=(h2 == HB // 2 - 1),
                                        perf_mode=DR)
                            for j in range(E // NCHUNK):
                                nc.vector.tensor_scalar_mul(
                                    t[:, j * NCHUNK:(j + 1) * NCHUNK],
                                    pss[j][:], C_F2 * S_H)
                            hres = pew.tile([P, E], BF16, tag="hres")
                            nc.sync.dma_start(hres[:],
                                              h_d[sb * P:(sb + 1) * P, :])
                            if not identity:
                                nc.vector.tensor_add(hres[:], hres[:], bf2_b[:])
                            stats = pew.tile([P, 2, 6], F32, tag="ln_stats")
                            for j in range(2):
                                sl = slice(j * 512, (j + 1) * 512)
                                nc.vector.tensor_add(t[:, sl], t[:, sl],
                                                     hres[:, sl])
                                nc.vector.bn_stats(stats[:, j, :], t[:, sl])
                            mv = pew.tile([P, 2], F32, tag="ln_mv")
                            nc.vector.bn_aggr(mv[:], stats[:])
                            std = pew.tile([P, 1], F32, tag="ln_std")
                            nc.scalar.activation(std[:], mv[:, 1:2], AF.Sqrt,
                                                 bias=eps_c[:], scale=1.0)
                            rstd = pew.tile([P, 1], F32, tag="ln_rstd")
                            nc.vector.reciprocal(rstd[:], std[:])
                            nc.vector.tensor_scalar(t[:], t[:], mv[:, 0:1],
                                                    rstd[:], ALU.subtract,
                                                    ALU.mult)
                            if not identity:
                                nc.vector.tensor_mul(t[:], t[:], g2_b[:])
                                nc.vector.tensor_add(t[:], t[:], b2_b[:])
                            nc.sync.dma_start(out_d[sb * P:(sb + 1) * P, :],
                                              t[:])

    nc.compile()
    return nc


def _get_nc(identity):
    if identity not in _CACHED_NC:
        _CACHED_NC[identity] = build_nc(identity)
    return _CACHED_NC[identity]


def kernel(**inputs):
    x = np.ascontiguousarray(np.asarray(inputs["x"], dtype=np.float32))
    B = x.shape[0]
    assert x.shape == (8, S, E), x.shape

    def q8(a, s):
        v = np.clip(np.asarray(a, np.float64) * s, -240.0, 240.0)
        return np.ascontiguousarray(v.astype(np.float32)
                                    .astype(ml_dtypes.float8_e4m3))

    def f32(a):
        return np.ascontiguousarray(np.asarray(a, dtype=np.float32))

    Wq = np.asarray(inputs["Wq"], np.float32)
    Wk = np.asarray(inputs["Wk"], np.float32)
    Wv = np.asarray(inputs["Wv"], np.float32)
    Wo = np.asarray(inputs["Wo"], np.float32)
    bq = np.asarray(inputs["bq"], np.float32)
    bk = np.asarray(inputs["bk"], np.float32)
    bv = np.asarray(inputs["bv"], np.float32)
    bo = np.asarray(inputs["bo"], np.float32)
    W1 = np.asarray(inputs["W1"], np.float32)
    W2 = np.asarray(inputs["W2"], np.float32)
    scale = np.float32(SCALE)

    M = Wq @ Wk.T
    NP_ = Wv @ Wo
    # shuffles: row p of Ms holds M[o*128+p, :] blocks concatenated over o
    Ms = q8(M.reshape(EB, P, E).transpose(1, 0, 2).reshape(P, EB * E), S_M)
    NPs = q8(NP_.reshape(EB, P, E).transpose(1, 0, 2).reshape(P, EB * E), S_NP)
    # W1s[c, p, t*E + ei*128 + j] = W1[ei*128+p, (4c+t)*128+j]
    W1s = q8(W1.reshape(EB, P, HB // 4, 4, P)
             .transpose(2, 1, 3, 0, 4).reshape(HB // 4, P, 4 * E), S_W1)
    W2s = q8(W2.reshape(HB, P, E).transpose(1, 0, 2).reshape(P, HB * E), S_W2)

    shared = {
        "Ms": Ms, "NPs": NPs, "W1s": W1s, "W2s": W2s,
        "bo2": f32(bo + bv @ Wo),
        "bf1": f32((np.asarray(inputs["bf1"], np.float32) * S_R)
                   .reshape(HB, P).T),
        "bf2": f32(inputs["bf2"]),
        "g1": f32(inputs["g1"]), "b1": f32(inputs["b1"]),
        "g2": f32(inputs["g2"]), "b2": f32(inputs["b2"]),
    }
    vq = Wk @ bq
    cq = float(bq @ bk)
    lse = np.float32(np.log(S_E))
    in_maps = [
        {"x": x[c],
         "wrow": f32((scale * (x[c] @ vq) + scale * cq + lse)
                     .reshape(SB, P).T),
         **shared}
        for c in range(B)
    ]

    identity = bool(
        np.all(np.asarray(inputs["g1"], np.float32) == 1.0)
        and np.all(np.asarray(inputs["b1"], np.float32) == 0.0)
        and np.all(np.asarray(inputs["g2"], np.float32) == 1.0)
        and np.all(np.asarray(inputs["b2"], np.float32) == 0.0)
        and np.all(shared["bo2"] == 0.0)
        and np.all(shared["bf2"] == 0.0)
    )
    nc = _get_nc(identity)
    trace = bool(int(os.environ.get("BERT_TRACE", "0")))
    res = run_bass_kernel_spmd(nc, in_maps, core_ids=list(range(B)), trace=trace)
    if trace and res.exec_time_ns is not None:
        print(f"HW exec time: {res.exec_time_ns} ns")
        kernel.last_exec_time_ns = res.exec_time_ns
        kernel.last_trace = res.instructions_and_trace
    return np.stack([res.results[c]["out"] for c in range(B)]).astype(np.float32)


# revision 35
# speedup vs baseline: 1.0640x; 1.0051x over previous
"""BERT encoder block on 8 Trainium2 NeuronCores.

Strategy: pure data parallelism — batch 8 is split one batch element per core
(no collectives). Each core runs the full encoder block on its [2048, 1024]
slice. All six big matmuls run in fp8 (TRN e4m3) DoubleRow mode (2 fp8
weights per PE cell -> 2x contraction per pass); accumulation and the
residual/LN stream are fp32.

Algebraic folds done on the host (softmax row-invariance absorbs the
query-side bias term):
  M   = Wq @ Wk^T          -> scores = x M x^T  (one fused tensor A = x@M)
  NP  = Wv @ Wo            -> attn_out @ Wo = softmax(S) @ (x@NP) + bo2
  bo2 = bo + bv @ Wo
  wrow = scale*(x @ (Wk@bq) + bq.bk) + ln(S_E)   (key-side score bias)

fp8 scaling: every fp8 tensor T is stored as s_T * T with a power-of-2
per-tensor scale chosen so absmax stays well under TRN e4m3's +-240 (and
above the 2^-6 subnormal threshold for typical values). All descales fold
into existing activation scale/bias parameters:
  xT   = S_X  * x^T            (scaled during the f32->bf16 pre-transpose copy)
  Ms   = S_M  * M, NPs = S_NP * NP, W1s = S_W1 * W1, W2s = S_W2 * W2  (host)
  AT   = S_A  * (x@M)^T        (PSUM copy scale = S_A/(S_M*S_X))
  VW   = S_VW * (x@NP)         (PSUM copy scale = S_VW/(S_X*S_NP))
  expS = S_E  * exp(..)        (ln S_E folded into wrow bias on host)
  proj = S_E * num             (PV psum copy scale 1/S_VW; the softmax
  normalization 1/(S_E*D) is applied per-row in d_chain on VectorE, which
  decouples the PV matmul stream from the denominator reduction)
  hT   = S_H  * h^T            (scaled during the f32->bf16 pre-transpose copy)
  r1   = S_R  * relu(..)       (relu scale = S_R/(S_W1*S_H), bias = S_R*bf1)
  FFN2 out descale = 1/(S_R*S_W2) via one tensor_scalar_mul.
The fp32 residual/LN stream is never scaled.

Attention runs in transposed score layout S^T[k,q]: softmax denominators are
ones-matmuls and proj = P @ (x@NP) lands directly in [q, f] layout. The
residual+LN1+h-transpose work is interleaved into the attention loop with a
one-chunk lag so TensorE never drains. The transposed LN1 output hT stays
resident in SBUF (no DRAM round-trip) and the FFN weights are prefetched on
the gpsimd DMA queue during phases A..C, so Phase E starts without stalls.
Weights are pre-shuffled on the host so every big DMA is one contiguous
segment per partition.

Self-contained: hardcodes shapes from the problem spec.
"""
import os

import numpy as np
import ml_dtypes

import concourse.bacc as bacc
import concourse.bass as bass
import concourse.tile as tile
import concourse.mybir as mybir
from concourse.bass_utils import run_bass_kernel_spmd
from concourse.masks import make_identity

P = 128
S = 2048          # sequence length per core
E = 1024          # embed
F = 4096          # ffn hidden
SB = S // P       # 16 seq blocks
EB = E // P       # 8 embed blocks
HB = F // P       # 32 ffn blocks
NCHUNK = 512
QC = S // NCHUNK  # 4 q chunks
QPC = NCHUNK // P  # 4 seq blocks per chunk
LN_EPS = 1e-5
SCALE = 1.0 / np.sqrt(np.float32(E))

# fp8 per-tensor scales (powers of 2; absmaxes measured on the fixed input
# distribution with ~2x safety margin under TRN e4m3's +-240)
S_X = 32.0    # x absmax ~5.4   -> 173
S_M = 2048.0  # M absmax ~.058  -> 118
S_NP = 2048.0  # NP absmax ~.053 -> 107
S_A = 32.0    # A absmax ~1.9   -> 62
S_VW = 64.0   # VW absmax ~1.8  -> 115
S_E = 16.0    # exp absmax ~7.5 -> 119
S_H = 32.0    # h absmax ~4.9   -> 158
S_W1 = 4096.0  # W1 absmax 1/32 -> 128
S_R = 16.0    # r1 absmax ~3.2  -> 50
S_W2 = 8192.0  # W2 absmax 1/64 -> 128

C_A = S_A / (S_M * S_X)        # AT copy scale
C_VW = S_VW / (S_X * S_NP)     # VW copy scale
C_EXP = float(SCALE) / (S_X * S_A)  # exp activation scale
C_R1 = S_R / (S_W1 * S_H)      # relu activation scale
C_F2 = 1.0 / (S_R * S_W2)      # FFN2 descale

F32 = mybir.dt.float32
BF16 = mybir.dt.bfloat16
F8 = mybir.dt.float8e4
AF = mybir.ActivationFunctionType
ALU = mybir.AluOpType
DR = mybir.MatmulPerfMode.DoubleRow

_CACHED_NC = {}


def _bcast_ap(ap, parts=P):
    """DRAM row-vector -> [parts, n] partition-broadcast access pattern."""
    return bass.AP(tensor=ap.tensor, offset=ap.offset,
                   ap=[[0, parts]] + [list(d) for d in ap.ap])


def _layer_norm_inplace(nc, work, src, gamma, beta, eps_c):
    """LN over free dim of src [P, E] fp32, in place.

    gamma/beta None means identity (fold for the common g=1, b=0 case).
    """
    stats = work.tile([P, 2, 6], F32, tag="ln_stats")
    nc.vector.bn_stats(stats[:, 0, :], src[:, 0:512])
    nc.vector.bn_stats(stats[:, 1, :], src[:, 512:1024])
    mv = work.tile([P, 2], F32, tag="ln_mv")
    nc.vector.bn_aggr(mv[:], stats[:])
    std = work.tile([P, 1], F32, tag="ln_std")
    nc.scalar.activation(std[:], mv[:, 1:2], AF.Sqrt, bias=eps_c[:], scale=1.0)
    rstd = work.tile([P, 1], F32, tag="ln_rstd")
    nc.vector.reciprocal(rstd[:], std[:])
    nc.vector.tensor_scalar(src[:], src[:], mv[:, 0:1], rstd[:],
                            ALU.subtract, ALU.mult)
    if gamma is not None:
        nc.vector.tensor_mul(src[:], src[:], gamma[:])
    if beta is not None:
        nc.vector.tensor_add(src[:], src[:], beta[:])


def build_nc(identity=False):
    """identity=True folds away LN gamma/beta and the bo2/bf2 bias adds,
    valid when g1=g2=1, b1=b2=bo2=bf2=0 (checked on the host)."""
    nc = bacc.Bacc(None, target_bir_lowering=False, debug=False)

    x_d = nc.dram_tensor("x", [S, E], F32, kind="ExternalInput")
    # host-preshuffled: row p holds M[o*128+p, :] for o in 0..7, concatenated
    m_d = nc.dram_tensor("Ms", [P, EB * E], F8, kind="ExternalInput")
    np_d = nc.dram_tensor("NPs", [P, EB * E], F8, kind="ExternalInput")
    # W1s[c, p, t*E + ei*128 + j] = W1[ei*128+p, (4c+t)*128+j]
    w1_d = nc.dram_tensor("W1s", [HB // 4, P, 4 * E], F8, kind="ExternalInput")
    # W2s[p, hb*E + n] = W2[hb*128+p, n]
    w2_d = nc.dram_tensor("W2s", [P, HB * E], F8, kind="ExternalInput")
    bo2_d = nc.dram_tensor("bo2", [E], F32, kind="ExternalInput")
    wrow_d = nc.dram_tensor("wrow", [P, SB], F32, kind="ExternalInput")
    bf1_d = nc.dram_tensor("bf1", [P, HB], F32, kind="ExternalInput")
    bf2_d = nc.dram_tensor("bf2", [E], F32, kind="ExternalInput")
    g1_d = nc.dram_tensor("g1", [E], F32, kind="ExternalInput")
    b1_d = nc.dram_tensor("b1", [E], F32, kind="ExternalInput")
    g2_d = nc.dram_tensor("g2", [E], F32, kind="ExternalInput")
    b2_d = nc.dram_tensor("b2", [E], F32, kind="ExternalInput")
    out_d = nc.dram_tensor("out", [S, E], F32, kind="ExternalOutput")
    h_d = nc.dram_tensor("h_scratch", [S, E], BF16)  # S_H * LN1 output spill

    with tile.TileContext(nc, pool_alloc_mode="queue") as tc:
        with tc.tile_pool(name="const", bufs=1) as const:
            ident = const.tile([P, P], BF16)
            make_identity(nc, ident)
            ones_c = const.tile([P, 1], F32)
            nc.vector.memset(ones_c[:], 1.0)
            eps_c = const.tile([P, 1], F32)
            nc.vector.memset(eps_c[:], LN_EPS)
            eps_h = const.tile([P, 1], F32)
            nc.vector.memset(eps_h[:], LN_EPS / (S_H * S_H))
            # keep these strided gathers off the sync queue so the first
            # x-tile DMA isn't stuck behind them
            bf1_sb = const.tile([P, HB], F32)
            nc.gpsimd.dma_start(bf1_sb[:], bf1_d[:])
            recip_sb = const.tile([P, SB], F32)
            w_sb = const.tile([P, SB], F32)
            nc.gpsimd.dma_start(w_sb[:], wrow_d[:])

            with tc.tile_pool(name="pres", bufs=1) as pres:
                # resident through phase E: transposed LN1 output (built in
                # phase C, consumed in phase E — never leaves SBUF) and the
                # FFN weights, prefetched on the gpsimd queue from t=0.
                hT_sb = pres.tile([P, EB, S], F8)   # S_H * h^T
                w1_sb = pres.tile([P, HB // 4, 4, EB, P], F8)

                with tc.tile_pool(name="pbig", bufs=1) as pbig:
                    xT = pbig.tile([P, EB, S], F8)  # S_X*x^T

                    # ---- Phase A: load x, scale, transpose to xT ----------
                    with tc.tile_pool(name="pa", bufs=3) as pa, \
                         tc.tile_pool(name="pa_ps", bufs=4, space="PSUM") as pa_ps:
                        for sb in range(SB):
                            xf = pa.tile([P, E], F32, tag="xf")
                            # alternate DMA queues: one queue alone can't
                            # sustain full HBM bandwidth for the x stream
                            q = nc.sync if sb % 2 == 0 else nc.gpsimd
                            q.dma_start(xf[:], x_d[sb * P:(sb + 1) * P, :])
                            xb = pa.tile([P, E], BF16, tag="xb")
                            nc.vector.tensor_scalar_mul(xb[:], xf[:], S_X)
                            for eb in range(EB):
                                pt = pa_ps.tile([P, P], BF16, tag="tp")
                                nc.tensor.transpose(
                                    pt[:], xb[:, eb * P:(eb + 1) * P], ident[:])
                                nc.scalar.copy(
                                    xT[:, eb, sb * P:(sb + 1) * P], pt[:])

                    with tc.tile_pool(name="pkv", bufs=1) as pkv:
                        AT = pkv.tile([P, EB, S], F8)   # S_A*(x@M)^T
                        VW = pkv.tile([P, SB, E], F8)   # S_VW*(x@NP), [k, f]

                        # ---- Phase B: AT, VW ------------------------------
                        with tc.tile_pool(name="wm", bufs=1) as wm, \
                             tc.tile_pool(name="pb_ps", bufs=4,
                                          space="PSUM") as pb_ps:
                            m_sb = wm.tile([P, EB, E], F8)
                            np_sb = wm.tile([P, EB, E], F8)
                            nc.gpsimd.dma_start(m_sb[:], m_d[:].rearrange(
                                "p (o n) -> p o n", n=E))
                            nc.gpsimd.dma_start(np_sb[:], np_d[:].rearrange(
                                "p (o n) -> p o n", n=E))
                            for eb in range(EB):
                                for qc in range(QC):
                                    ps = pb_ps.tile([P, NCHUNK], F32, tag="mm")
                                    for e2 in range(EB // 2):
                                        nc.tensor.matmul(
                                            ps[:],
                                            m_sb[:, 2 * e2:2 * e2 + 2,
                                                 eb * P:(eb + 1) * P],
                                            xT[:, 2 * e2:2 * e2 + 2,
                                               qc * NCHUNK:(qc + 1) * NCHUNK],
                                            start=(e2 == 0),
                                            stop=(e2 == EB // 2 - 1),
                                            perf_mode=DR)
                                    nc.scalar.activation(
                                        AT[:, eb, qc * NCHUNK:(qc + 1) * NCHUNK],
                                        ps[:], AF.Copy, scale=C_A)
                            for sb in range(SB):
                                ps_a = pb_ps.tile([P, NCHUNK], F32, tag="mm")
                                ps_b = pb_ps.tile([P, NCHUNK], F32, tag="mm")
                                pss = [ps_a, ps_b]
                                for e2 in range(EB // 2):
                                    for ec in range(E // NCHUNK):
                                        nc.tensor.matmul(
                                            pss[ec][:],
                                            xT[:, 2 * e2:2 * e2 + 2,
                                               sb * P:(sb + 1) * P],
                                            np_sb[:, 2 * e2:2 * e2 + 2,
                                                  ec * NCHUNK:(ec + 1) * NCHUNK],
                                            start=(e2 == 0),
                                            stop=(e2 == EB // 2 - 1),
                                            perf_mode=DR)
                                for ec in range(E // NCHUNK):
                                    nc.scalar.activation(
                                        VW[:, sb, ec * NCHUNK:(ec + 1) * NCHUNK],
                                        pss[ec][:], AF.Copy, scale=C_VW)

                        # ---- Phase C: attention + proj, LN1 interleaved ---
                        with tc.tile_pool(name="pexp", bufs=2) as pexp, \
                             tc.tile_pool(name="pcw", bufs=1) as pcw, \
                             tc.tile_pool(name="pproj", bufs=2 * QPC) as pproj, \
                             tc.tile_pool(name="lnc", bufs=1) as lnc, \
                             tc.tile_pool(name="pdw", bufs=2) as pdw, \
                             tc.tile_pool(name="pc_ps", bufs=3,
                                          space="PSUM") as pc_ps, \
                             tc.tile_pool(name="pp_ps", bufs=2,
                                          space="PSUM") as pp_ps, \
                             tc.tile_pool(name="pr_ps", bufs=1,
                                          space="PSUM") as pr_ps, \
                             tc.tile_pool(name="pdt_ps", bufs=2,
                                          space="PSUM") as pdt_ps:
                            # W1 prefetch streams during phase C (issuing
                            # it earlier would steal DMA bandwidth from the
                            # phase-A x loads)
                            for c in range(HB // 4):
                                nc.gpsimd.dma_start(
                                    w1_sb[:, c], w1_d[c].rearrange(
                                        "p (t o n) -> p t o n", t=4, o=EB))
                            if identity:
                                bo2_b = g1_b = b1_b = None
                            else:
                                bo2_b = lnc.tile([P, E], F32)
                                g1_b = lnc.tile([P, E], F32)
                                b1_b = lnc.tile([P, E], F32)
                                nc.sync.dma_start(bo2_b[:], _bcast_ap(bo2_d[:]))
                                nc.sync.dma_start(g1_b[:], _bcast_ap(g1_d[:]))
                                nc.sync.dma_start(b1_b[:], _bcast_ap(b1_d[:]))
                                nc.vector.tensor_scalar_mul(g1_b[:], g1_b[:],
                                                            S_H)
                                nc.vector.tensor_scalar_mul(b1_b[:], b1_b[:],
                                                            S_H)
                            proj_tiles = {}

                            def d_chain(sb):
                                """residual + LN1 + transpose, one seq block."""
                                xf = pdw.tile([P, E], F32, tag="xres")
                                nc.sync.dma_start(xf[:],
                                                  x_d[sb * P:(sb + 1) * P, :])
                                hpre = pdw.tile([P, E], F32, tag="hpre")
                                nc.vector.tensor_scalar(hpre[:],
                                                        proj_tiles.pop(sb)[:],
                                                        recip_sb[:, sb:sb + 1],
                                                        None, ALU.mult)
                                if not identity:
                                    nc.vector.tensor_add(hpre[:], hpre[:],
                                                         bo2_b[:])
                                nc.vector.tensor_add(hpre[:], hpre[:], xf[:])
                                # LN1 with S_H folded in: Sqrt activation
                                # scale gives std/S_H, so the final
                                # normalize emits bf16 S_H*h in one op.
                                stats = pdw.tile([P, 2, 6], F32, tag="ln_stats")
                                nc.vector.bn_stats(stats[:, 0, :],
                                                   hpre[:, 0:512])
                                nc.vector.bn_stats(stats[:, 1, :],
                                                   hpre[:, 512:1024])
                                mv = pdw.tile([P, 2], F32, tag="ln_mv")
                                nc.vector.bn_aggr(mv[:], stats[:])
                                std = pdw.tile([P, 1], F32, tag="ln_std")
                                nc.scalar.activation(std[:], mv[:, 1:2], AF.Sqrt,
                                                     bias=eps_h[:],
                                                     scale=1.0 / (S_H * S_H))
                                rstd = pdw.tile([P, 1], F32, tag="ln_rstd")
                                nc.vector.reciprocal(rstd[:], std[:])
                                hb16 = pdw.tile([P, E], BF16, tag="hb16")
                                if identity:
                                    nc.vector.tensor_scalar(hb16[:], hpre[:],
                                                            mv[:, 0:1], rstd[:],
                                                            ALU.subtract,
                                                            ALU.mult)
                                else:
                                    nc.vector.tensor_scalar(hpre[:], hpre[:],
                                                            mv[:, 0:1], rstd[:],
                                                            ALU.subtract,
                                                            ALU.mult)
                                    nc.vector.tensor_mul(hpre[:], hpre[:],
                                                         g1_b[:])
                                    nc.vector.tensor_add(hb16[:], hpre[:],
                                                         b1_b[:])
                                nc.sync.dma_start(h_d[sb * P:(sb + 1) * P, :],
                                                  hb16[:])
                                for eb in range(EB):
                                    pt = pdt_ps.tile([P, P], BF16, tag="tp2")
                                    nc.tensor.transpose(
                                        pt[:], hb16[:, eb * P:(eb + 1) * P],
                                        ident[:])
                                    nc.scalar.copy(
                                        hT_sb[:, eb, sb * P:(sb + 1) * P], pt[:])

                            for qc in range(QC):
                                expS = pexp.tile([P, SB, NCHUNK], F8, tag="expS")
                                for kb in range(SB):
                                    ps = pc_ps.tile([P, NCHUNK], F32, tag="s")
                                    for e2 in range(EB // 2):
                                        nc.tensor.matmul(
                                            ps[:],
                                            xT[:, 2 * e2:2 * e2 + 2,
                                               kb * P:(kb + 1) * P],
                                            AT[:, 2 * e2:2 * e2 + 2,
                                               qc * NCHUNK:(qc + 1) * NCHUNK],
                                            start=(e2 == 0),
                                            stop=(e2 == EB // 2 - 1),
                                            perf_mode=DR)
                                    nc.scalar.activation(
                                        expS[:, kb, :], ps[:], AF.Exp,
                                        bias=w_sb[:, kb:kb + 1], scale=C_EXP)
                                if qc > 0:
                                    for qs in range(QPC):
                                        d_chain((qc - 1) * QPC + qs)
                                acc = [None] * 8
                                for j in range(8):
                                    a = pcw.tile([P, NCHUNK], F32, tag=f"acc{j}")
                                    nc.vector.tensor_add(a[:], expS[:, j, :],
                                                         expS[:, j + 8, :])
                                    acc[j] = a
                                for j in range(4):
                                    nc.vector.tensor_add(acc[j][:], acc[j][:],
                                                         acc[j + 4][:])
                                for j in range(2):
                                    nc.vector.tensor_add(acc[j][:], acc[j][:],
                                                         acc[j + 2][:])
                                nc.vector.tensor_add(acc[0][:], acc[0][:],
                                                     acc[1][:])
                                for qs in range(QPC):
                                    sb = qc * QPC + qs
                                    proj = pproj.tile([P, E], BF16, tag="proj")
                                    proj_tiles[sb] = proj
                                    for fc in range(E // NCHUNK):
                                        ps = pp_ps.tile([P, NCHUNK], F32,
                                                        tag="pp")
                                        for k2 in range(SB // 2):
                                            nc.tensor.matmul(
                                                ps[:],
                                                expS[:, 2 * k2:2 * k2 + 2,
                                                     qs * P:(qs + 1) * P],
                                                VW[:, 2 * k2:2 * k2 + 2,
                                                   fc * NCHUNK:(fc + 1) * NCHUNK],
                                                start=(k2 == 0),
                                                stop=(k2 == SB // 2 - 1),
                                                perf_mode=DR)
                                        nc.scalar.activation(
                                            proj[:, fc * NCHUNK:(fc + 1) * NCHUNK],
                                            ps[:], AF.Copy, scale=1.0 / S_VW)
                                for qs in range(QPC):
                                    sb = qc * QPC + qs
                                    pr = pr_ps.tile([P, 1], F32, tag="rs")
                                    nc.tensor.matmul(
                                        pr[:], acc[0][:, qs * P:(qs + 1) * P],
                                        ones_c[:], start=True, stop=True)
                                    nc.vector.reciprocal(recip_sb[:, sb:sb + 1],
                                                         pr[:])
                            for qs in range(QPC):
                                d_chain((QC - 1) * QPC + qs)
                    # pkv, pbig closed

                # ---- Phase E: FFN + LN2 + out -----------------------------
                with tc.tile_pool(name="w2r", bufs=1) as w2r, \
                     tc.tile_pool(name="lnc2", bufs=1) as lnc2, \
                     tc.tile_pool(name="pr1a", bufs=2) as pr1a, \
                     tc.tile_pool(name="pew", bufs=3) as pew, \
                     tc.tile_pool(name="pr1_ps", bufs=3, space="PSUM") as pr1_ps, \
                     tc.tile_pool(name="pf2_ps", bufs=4, space="PSUM") as pf2_ps:
                    w2_sb = w2r.tile([P, HB, E], F8)
                    w2_r = w2_d[:].rearrange("p (o n) -> p o n", n=E)
                    for hq in range(4):
                        nc.gpsimd.dma_start(
                            w2_sb[:, hq * (HB // 4):(hq + 1) * (HB // 4), :],
                            w2_r[:, hq * (HB // 4):(hq + 1) * (HB // 4), :])
                    if identity:
                        bf2_b = g2_b = b2_b = None
                    else:
                        bf2_b = lnc2.tile([P, E], F32)
                        g2_b = lnc2.tile([P, E], F32)
                        b2_b = lnc2.tile([P, E], F32)
                        nc.sync.dma_start(bf2_b[:], _bcast_ap(bf2_d[:]))
                        nc.vector.tensor_scalar_mul(bf2_b[:], bf2_b[:], S_H)
                        nc.sync.dma_start(g2_b[:], _bcast_ap(g2_d[:]))
                        nc.sync.dma_start(b2_b[:], _bcast_ap(b2_d[:]))
                    QW = 4 * P  # 4 seq blocks per group
                    for g in range(S // QW):
                        r1_all = pr1a.tile([P, HB, QW], F8, tag="r1a")
                        for c in range(HB // 4):
                            for t in range(4):
                                hb = c * 4 + t
                                ps1 = pr1_ps.tile([P, QW], F32, tag="r1")
                                for e2 in range(EB // 2):
                                    nc.tensor.matmul(
                                        ps1[:],
                                        w1_sb[:, c, t, 2 * e2:2 * e2 + 2, :],
                                        hT_sb[:, 2 * e2:2 * e2 + 2,
                                              g * QW:(g + 1) * QW],
                                        start=(e2 == 0),
                                        stop=(e2 == EB // 2 - 1),
                                        perf_mode=DR)
                                nc.scalar.activation(r1_all[:, hb, :], ps1[:],
                                                     AF.Relu,
                                                     bias=bf1_sb[:, hb:hb + 1],
                                                     scale=C_R1)
                        for i in range(QW // P):
                            sb = g * (QW // P) + i
                            t = pew.tile([P, E], F32, tag="ffn")
                            ps_a = pf2_ps.tile([P, NCHUNK], F32, tag="f2")
                            ps_b = pf2_ps.tile([P, NCHUNK], F32, tag="f2")
                            pss = [ps_a, ps_b]
                            for h2 in range(HB // 2):
                                for j in range(E // NCHUNK):
                                    nc.tensor.matmul(
                                        pss[j][:],
                                        r1_all[:, 2 * h2:2 * h2 + 2,
                                               i * P:(i + 1) * P],
                                        w2_sb[:, 2 * h2:2 * h2 + 2,
                                              j * NCHUNK:(j + 1) * NCHUNK],
                                        start=(h2 == 0),
                                        stop=(h2 == HB // 2 - 1),
                                        perf_mode=DR)
                            for j in range(E // NCHUNK):
                                nc.vector.tensor_scalar_mul(
                                    t[:, j * NCHUNK:(j + 1) * NCHUNK],
                                    pss[j][:], C_F2 * S_H)
                            hres = pew.tile([P, E], BF16, tag="hres")
                            nc.sync.dma_start(hres[:],
                                              h_d[sb * P:(sb + 1) * P, :])
                            if not identity:
                                nc.vector.tensor_add(hres[:], hres[:], bf2_b[:])
                            stats = pew.tile([P, 2, 6], F32, tag="ln_stats")
                            for j in range(2):
                                sl = slice(j * 512, (j + 1) * 512)
                                nc.vector.tensor_add(t[:, sl], t[:, sl],
                                                     hres[:, sl])
                                nc.vector.bn_stats(stats[:, j, :], t[:, sl])
                            mv = pew.tile([P, 2], F32, tag="ln_mv")
                            nc.vector.bn_aggr(mv[:], stats[:])
                            std = pew.tile([P, 1], F32, tag="ln_std")
                            nc.scalar.activation(std[:], mv[:, 1:2], AF.Sqrt,
                                                 bias=eps_c[:], scale=1.0)
                            rstd = pew.tile([P, 1], F32, tag="ln_rstd")
                            nc.vector.reciprocal(rstd[:], std[:])
                            nc.vector.tensor_scalar(t[:], t[:], mv[:, 0:1],
                                                    rstd[:], ALU.subtract,
                                                    ALU.mult)
                            if not identity:
                                nc.vector.tensor_mul(t[:], t[:], g2_b[:])
                                nc.vector.tensor_add(t[:], t[:], b2_b[:])
                            nc.sync.dma_start(out_d[sb * P:(sb + 1) * P, :],
                                              t[:])

    nc.compile()
    return nc


def _get_nc(identity):
    if identity not in _CACHED_NC:
        _CACHED_NC[identity] = build_nc(identity)
    return _CACHED_NC[identity]


def kernel(**inputs):
    x = np.ascontiguousarray(np.asarray(inputs["x"], dtype=np.float32))
    B = x.shape[0]
    assert x.shape == (8, S, E), x.shape

    def q8(a, s):
        v = np.clip(np.asarray(a, np.float64) * s, -240.0, 240.0)
        return np.ascontiguousarray(v.astype(np.float32)
                                    .astype(ml_dtypes.float8_e4m3))

    def f32(a):
        return np.ascontiguousarray(np.asarray(a, dtype=np.float32))

    Wq = np.asarray(inputs["Wq"], np.float32)
    Wk = np.asarray(inputs["Wk"], np.float32)
    Wv = np.asarray(inputs["Wv"], np.float32)
    Wo = np.asarray(inputs["Wo"], np.float32)
    bq = np.asarray(inputs["bq"], np.float32)
    bk = np.asarray(inputs["bk"], np.float32)
    bv = np.asarray(inputs["bv"], np.float32)
    bo = np.asarray(inputs["bo"], np.float32)
    W1 = np.asarray(inputs["W1"], np.float32)
    W2 = np.asarray(inputs["W2"], np.float32)
    scale = np.float32(SCALE)

    M = Wq @ Wk.T
    NP_ = Wv @ Wo
    # shuffles: row p of Ms holds M[o*128+p, :] blocks concatenated over o
    Ms = q8(M.reshape(EB, P, E).transpose(1, 0, 2).reshape(P, EB * E), S_M)
    NPs = q8(NP_.reshape(EB, P, E).transpose(1, 0, 2).reshape(P, EB * E), S_NP)
    # W1s[c, p, t*E + ei*128 + j] = W1[ei*128+p, (4c+t)*128+j]
    W1s = q8(W1.reshape(EB, P, HB // 4, 4, P)
             .transpose(2, 1, 3, 0, 4).reshape(HB // 4, P, 4 * E), S_W1)
    W2s = q8(W2.reshape(HB, P, E).transpose(1, 0, 2).reshape(P, HB * E), S_W2)

    shared = {
        "Ms": Ms, "NPs": NPs, "W1s": W1s, "W2s": W2s,
        "bo2": f32(bo + bv @ Wo),
        "bf1": f32((np.asarray(inputs["bf1"], np.float32) * S_R)
                   .reshape(HB, P).T),
        "bf2": f32(inputs["bf2"]),
        "g1": f32(inputs["g1"]), "b1": f32(inputs["b1"]),
        "g2": f32(inputs["g2"]), "b2": f32(inputs["b2"]),
    }
    vq = Wk @ bq
    cq = float(bq @ bk)
    lse = np.float32(np.log(S_E))
    in_maps = [
        {"x": x[c],
         "wrow": f32((scale * (x[c] @ vq) + scale * cq + lse)
                     .reshape(SB, P).T),
         **shared}
        for c in range(B)
    ]

    identity = bool(
        np.all(np.asarray(inputs["g1"], np.float32) == 1.0)
        and np.all(np.asarray(inputs["b1"], np.float32) == 0.0)
        and np.all(np.asarray(inputs["g2"], np.float32) == 1.0)
        and np.all(np.asarray(inputs["b2"], np.float32) == 0.0)
        and np.all(shared["bo2"] == 0.0)
        and np.all(shared["bf2"] == 0.0)
    )
    nc = _get_nc(identity)
    trace = bool(int(os.environ.get("BERT_TRACE", "0")))
    res = run_bass_kernel_spmd(nc, in_maps, core_ids=list(range(B)), trace=trace)
    if trace and res.exec_time_ns is not None:
        print(f"HW exec time: {res.exec_time_ns} ns")
        kernel.last_exec_time_ns = res.exec_time_ns
        kernel.last_trace = res.instructions_and_trace
    return np.stack([res.results[c]["out"] for c in range(B)]).astype(np.float32)
